# revision 1
# baseline (speedup 1.0000x reference)
"""ClusterAttention Trainium2 kernel.

Sharding: 48 (b*h) rows -> 6 rows per core (8 cores). Host gathers feat into
cluster order per row (transposed, c-major) as shard prep; device does all
matmul/softmax compute; host scatters head outputs back to token order between
the two device phases and sums nothing (phase B consumes all 12 heads per
token on one core).

Phase A (per core, per row r with head h):
  - o-major qk projection: psum[52,512] = wqk[cc].T @ featT tile, accumulated
    over 3 c-chunks. M-layout: rows 0:16 q*scale, 16 qA, 32:48 k, 51 kB.
  - q_sb[20,8192] rows: [q(16), qA, 1, -s, 1];  k_sb rows: [k(16), 1, s+b+c, 1, kB]
    so that sum_p q_aug[p]*k_aug[p] = scale*(q+bq).(k+bk) + s_j - s_i + b_pos.
  - t-major v projection: psum[128,256] = featT_chunk.T @ wv[cc], -> v_sb with a
    ones column per 64-col chunk (for the softmax denominator).
  - per cluster: S'[j,i] = k_aug.T@q_aug (K=20), E = exp(S'), AV: out[i,0:65] =
    sum_j E[j,i-chunk].T @ v_aug[j,0:65]; col 64 = denominator. Normalize by
    DVE reciprocal + per-partition scalar multiply. DMA out rows.

Phase B (per core): o-major projection outT[o,t] = w_proj chunks.T @ feat2T,
bias added via DVE tensor_scalar.
"""
import os
import numpy as np
import ml_dtypes

import concourse.bacc as bacc
import concourse.tile as tile
from concourse import mybir
from concourse.bass_utils import run_bass_kernel_spmd

B, N, C, H, D, K, M = 4, 8192, 384, 12, 2, 32, 256
CH = C // H // 2            # 16
BH = B * H                  # 48
R = BH // 8                 # 6 rows per core
SCALE = float((C // H) ** -0.5)
NT = N // 512               # 16 token tiles per row
TPB = N * B // 8            # 4096 tokens per core in phase B

_DT = {"f32": mybir.dt.float32, "bf16": mybir.dt.bfloat16, "f32r": mybir.dt.float32r}
_NP = {"f32": np.float32, "bf16": ml_dtypes.bfloat16, "f32r": np.float32}


def _parse_cfg():
    s = os.environ.get("KCFG", "feat=f32,qk=f32,e=f32,b=f32,r=0")
    cfg = {}
    for part in s.split(","):
        k, v = part.split("=")
        cfg[k] = v
    cfg.setdefault("feat", "f32"); cfg.setdefault("qk", "f32")
    cfg.setdefault("e", "f32"); cfg.setdefault("b", "f32")
    cfg.setdefault("r", "0")  # r=1: bitcast f32 matmul operands to float32r
    return cfg


CFG = _parse_cfg()


def _mm(ap, kind):
    # bitcast fp32 matmul inputs to float32r when enabled
    if CFG["r"] == "1" and CFG[kind] == "f32":
        return ap.bitcast(mybir.dt.float32r)
    return ap


def build_phase_a():
    dt_feat, dt_qk, dt_e = _DT[CFG["feat"]], _DT[CFG["qk"]], _DT[CFG["e"]]
    dt_wv = mybir.dt.float32 if CFG["feat"] == "f32r" else dt_feat
    nc = bacc.Bacc(None, target_bir_lowering=False)
    featT = nc.dram_tensor("featT", [R * 3 * 128, N], dt_feat, kind="ExternalInput")
    wqk = nc.dram_tensor("wqk", [R * 3 * 128, 52], dt_feat, kind="ExternalInput")
    wv = nc.dram_tensor("wv", [R * 3 * 128, 64], dt_wv, kind="ExternalInput")
    aux = nc.dram_tensor("aux", [R * 8, N], dt_qk, kind="ExternalInput")
    out_g = nc.dram_tensor("out_g", [R * N, 64], mybir.dt.float32, kind="ExternalOutput")

    with tile.TileContext(nc) as tc:
        with (
            tc.tile_pool(name="sb_feat", bufs=1) as p_feat,
            tc.tile_pool(name="sb_row", bufs=1) as p_row,
            tc.tile_pool(name="sb_w", bufs=2) as p_w,
            tc.tile_pool(name="sb_e", bufs=4) as p_e,
            tc.tile_pool(name="sb_out", bufs=4) as p_out,
            tc.tile_pool(name="ps_qkp", bufs=2, space="PSUM") as ps_qkp,
            tc.tile_pool(name="ps_sp", bufs=2, space="PSUM") as ps_sp,
            tc.tile_pool(name="ps_vp", bufs=2, space="PSUM") as ps_vp,
            tc.tile_pool(name="ps_op", bufs=2, space="PSUM") as ps_op,
        ):
            for r in range(R):
                fb = r * 3 * 128
                # per-row persistent tiles
                ft = p_feat.tile([128, 3 * N], dt_feat, tag="ft")
                q_sb = p_row.tile([20, N], dt_qk, tag="q_sb")
                k_sb = p_row.tile([20, N], dt_qk, tag="k_sb")
                v_sb = p_row.tile([128, 64 * 65], dt_e, tag="v_sb")
                wqk_sb = p_w.tile([128, 3 * 52], dt_feat, tag="wqk_sb")
                wv_sb = p_w.tile([128, 3 * 64], dt_wv, tag="wv_sb")
                for cc in range(3):
                    nc.sync.dma_start(ft[:, cc * N:(cc + 1) * N],
                                      featT[fb + cc * 128: fb + (cc + 1) * 128, :])
                    nc.sync.dma_start(wqk_sb[:, cc * 52:(cc + 1) * 52],
                                      wqk[fb + cc * 128: fb + (cc + 1) * 128, :])
                    nc.sync.dma_start(wv_sb[:, cc * 64:(cc + 1) * 64],
                                      wv[fb + cc * 128: fb + (cc + 1) * 128, :])
                # host aux rows: q rows 16:20 = [0, 1, -s', 1]; k rows 16:20 = [1, s+b+c', 1, 0]
                nc.sync.dma_start(q_sb[16:20, :], aux[r * 8 + 0: r * 8 + 4, :])
                nc.sync.dma_start(k_sb[16:20, :], aux[r * 8 + 4: r * 8 + 8, :])
                v_view = v_sb.rearrange("p (c w) -> p c w", w=65)
                nc.vector.memset(v_view[:, :, 64], 1.0)

                for tt in range(NT):
                    t0 = tt * 512
                    # --- qk projection (o-major), psum rows: q 0:17, k 32:52
                    ps_qk = ps_qkp.tile([52, 512], mybir.dt.float32, tag="ps_qk")
                    for cc in range(3):
                        nc.tensor.matmul(
                            ps_qk[:, :],
                            _mm(wqk_sb[:, cc * 52:(cc + 1) * 52], "feat"),
                            _mm(ft[:, cc * N + t0: cc * N + t0 + 512], "feat"),
                            start=(cc == 0), stop=(cc == 2))
                    nc.vector.tensor_copy(q_sb[0:16, t0:t0 + 512], ps_qk[0:16, :])
                    nc.vector.tensor_copy(k_sb[0:16, t0:t0 + 512], ps_qk[32:48, :])
                    # --- v projection (t-major), 4 chunks of 128 tokens
                    ps_v = ps_vp.tile([128, 256], mybir.dt.float32, tag="ps_v")
                    for u in range(4):
                        tc0 = t0 + u * 128
                        for cc in range(3):
                            ftv = ft[:, cc * N + tc0: cc * N + tc0 + 128]
                            if CFG["feat"] == "f32r":
                                ftv = ftv.bitcast(mybir.dt.float32)
                            nc.tensor.matmul(
                                ps_v[:, u * 64:(u + 1) * 64],
                                _mm(ftv, "feat"),
                                _mm(wv_sb[:, cc * 64:(cc + 1) * 64], "feat"),
                                start=(u == 0 and cc == 0), stop=(u == 3 and cc == 2))
                    ci0 = tt * 4
                    nc.vector.tensor_copy(
                        v_view[:, ci0:ci0 + 4, 0:64],
                        ps_v.rearrange("p (c w) -> p c w", w=64))

                # --- attention: cluster pairs
                for pp in range(K // 2):
                    kk0 = pp * 2
                    e_tiles = []
                    for jc in range(2):
                        ps_s = ps_sp.tile([128, 512], mybir.dt.float32, tag="ps_s")
                        for u in range(2):  # cluster kk0+u
                            col = (kk0 + u) * 256
                            nc.tensor.matmul(
                                ps_s[:, u * 256:(u + 1) * 256],
                                _mm(k_sb[0:20, col + jc * 128: col + (jc + 1) * 128], "qk"),
                                _mm(q_sb[0:20, col: col + 256], "qk"),
                                start=(u == 0), stop=(u == 1))
                        et = p_e.tile([128, 512], dt_e, tag="e")
                        nc.scalar.activation(et[:, :], ps_s[:, :],
                                             mybir.ActivationFunctionType.Exp)
                        e_tiles.append(et)
                    for u in range(2):
                        kk = kk0 + u
                        for ic in range(2):
                            ps_o = ps_op.tile([128, 65], mybir.dt.float32, tag="ps_o")
                            for jc in range(2):
                                nc.tensor.matmul(
                                    ps_o[:, :],
                                    _mm(e_tiles[jc][:, u * 256 + ic * 128:
                                                    u * 256 + (ic + 1) * 128], "e"),
                                    _mm(v_view[:, kk * 2 + jc, :], "e"),
                                    start=(jc == 0), stop=(jc == 1))
                            inv = p_out.tile([128, 1], mybir.dt.float32, tag="inv")
                            nc.vector.reciprocal(inv[:, :], ps_o[:, 64:65])
                            ot = p_out.tile([128, 64], mybir.dt.float32, tag="ot")
                            nc.vector.tensor_scalar_mul(ot[:, :], ps_o[:, 0:64], inv[:, :])
                            row0 = r * N + kk * 256 + ic * 128
                            nc.sync.dma_start(out_g[row0: row0 + 128, :], ot[:, :])
    nc.compile()
    return nc


def build_phase_b():
    dt_b = _DT[CFG["b"]]
    nc = bacc.Bacc(None, target_bir_lowering=False)
    f2T = nc.dram_tensor("f2T", [6 * 128, TPB], dt_b, kind="ExternalInput")
    wp2 = nc.dram_tensor("wp2", [6 * 128, 384], dt_b, kind="ExternalInput")
    bias = nc.dram_tensor("bias", [3 * 128, 1], mybir.dt.float32, kind="ExternalInput")
    outT = nc.dram_tensor("outT", [3 * 128, TPB], mybir.dt.float32, kind="ExternalOutput")

    with tile.TileContext(nc) as tc:
        with (
            tc.tile_pool(name="sb", bufs=1) as pool,
            tc.tile_pool(name="sb_o", bufs=4) as p_o,
            tc.tile_pool(name="ps", bufs=4, space="PSUM") as ps,
        ):
            fsb = pool.tile([128, 6 * TPB], dt_b, tag="fsb")
            wsb = pool.tile([128, 6 * 384], dt_b, tag="wsb")
            bsb = pool.tile([128, 3], mybir.dt.float32, tag="bsb")
            for cc in range(6):
                nc.sync.dma_start(fsb[:, cc * TPB:(cc + 1) * TPB],
                                  f2T[cc * 128:(cc + 1) * 128, :])
                nc.sync.dma_start(wsb[:, cc * 384:(cc + 1) * 384],
                                  wp2[cc * 128:(cc + 1) * 128, :])
            for oc in range(3):
                nc.sync.dma_start(bsb[:, oc:oc + 1], bias[oc * 128:(oc + 1) * 128, :])
            for tt in range(TPB // 512):
                t0 = tt * 512
                for oc in range(3):
                    p = ps.tile([128, 512], mybir.dt.float32, tag="p")
                    for cc in range(6):
                        nc.tensor.matmul(
                            p[:, :],
                            _mm(wsb[:, cc * 384 + oc * 128: cc * 384 + (oc + 1) * 128], "b"),
                            _mm(fsb[:, cc * TPB + t0: cc * TPB + t0 + 512], "b"),
                            start=(cc == 0), stop=(cc == 5))
                    osb = p_o.tile([128, 512], mybir.dt.float32, tag="osb")
                    nc.vector.tensor_scalar(osb[:, :], p[:, :], bsb[:, oc:oc + 1], None,
                                            mybir.AluOpType.add)
                    nc.sync.dma_start(outT[oc * 128:(oc + 1) * 128, t0:t0 + 512],
                                      osb[:, :])
    nc.compile()
    return nc


_CACHE = {}


def _get(name, builder):
    if name not in _CACHE:
        _CACHE[name] = builder()
    return _CACHE[name]


def kernel(pos, feat, member_idx, w_qkv, b_qkv, w_pos, b_pos, w_proj, b_proj):
    import time
    np_feat, np_qk, np_e, np_b = (_NP[CFG[k]] for k in ("feat", "qk", "e", "b"))
    pos = np.asarray(pos, np.float32)
    feat = np.asarray(feat, np.float32)
    mf = np.asarray(member_idx).astype(np.int64).reshape(BH, N)
    w_qkv = np.asarray(w_qkv, np.float32); b_qkv = np.asarray(b_qkv, np.float32)
    w_pos = np.asarray(w_pos, np.float32); b_pos = np.asarray(b_pos, np.float32)
    w_proj = np.asarray(w_proj, np.float32); b_proj = np.asarray(b_proj, np.float32)

    t_prep0 = time.time()
    # ---- host shard prep
    pos_n = pos / pos.reshape(-1, D).max(0)
    b_of = np.repeat(np.arange(B), H)
    pos_g = np.take_along_axis(pos_n[b_of], mf[:, :, None], axis=1)      # [48,N,2]
    s_all = np.einsum('rnd,rd->rn', pos_g, np.tile(w_pos, (B, 1)))       # [48,N]

    featbig = np.ascontiguousarray(feat.transpose(0, 2, 1))              # [B,C,N]
    featT_all = np.empty((BH, C, N), np.float32)
    for r in range(BH):
        np.take(featbig[b_of[r]], mf[r], axis=1, out=featT_all[r])

    wqk_all = np.zeros((BH, C, 52), np.float32)
    wv_all = np.empty((BH, C, 64), np.float32)
    aux_all = np.empty((BH, 8, N), np.float32)
    for h in range(H):
        Wq = w_qkv[h * 96: h * 96 + 16]
        Wk = w_qkv[h * 96 + 16: h * 96 + 32]
        Wv = w_qkv[h * 96 + 32: h * 96 + 96]
        bq = b_qkv[h * 96: h * 96 + 16]
        bk = b_qkv[h * 96 + 16: h * 96 + 32]
        has_bias = bool(np.any(bq) or np.any(bk))
        for b in range(B):
            r = b * H + h
            wqk_all[r, :, 0:16] = SCALE * Wq.T
            wqk_all[r, :, 32:48] = Wk.T
            wv_all[r] = Wv.T
            aux_all[r, 0] = 0.0
            aux_all[r, 1] = 1.0
            aux_all[r, 2] = -s_all[r]
            aux_all[r, 3] = 1.0
            aux_all[r, 4] = 1.0
            aux_all[r, 5] = s_all[r] + b_pos[h]
            aux_all[r, 6] = 1.0
            aux_all[r, 7] = 0.0
            if has_bias:
                # exact bias folding: logit = scale*(q+bq).(k+bk) + ...
                q_raw = Wq @ featT_all[r].reshape(C, N)
                k_raw = Wk @ featT_all[r].reshape(C, N)
                aux_all[r, 2] += SCALE * (bk @ q_raw)
                aux_all[r, 5] += SCALE * (bq @ k_raw) + SCALE * float(bq @ bk)

    in_maps_a = []
    for c in range(8):
        rs = slice(c * R, (c + 1) * R)
        in_maps_a.append({
            "featT": featT_all[rs].reshape(R * 3 * 128, N).astype(np_feat, copy=False),
            "wqk": wqk_all[rs].reshape(R * 3 * 128, 52).astype(np_feat, copy=False),
            "wv": wv_all[rs].reshape(R * 3 * 128, 64).astype(np_feat, copy=False),
            "aux": aux_all[rs].reshape(R * 8, N).astype(np_qk, copy=False),
        })
    t_prep1 = time.time()

    nc_a = _get("a", build_phase_a)
    t_run_a0 = time.time()
    res_a = run_bass_kernel_spmd(nc_a, in_maps_a, core_ids=list(range(8)))
    t_run_a1 = time.time()

    out_g_all = np.concatenate(
        [res_a.results[c]["out_g"].reshape(R, N, 64) for c in range(8)], axis=0)

    # ---- host scatter to token order, build feat2T
    out_gT = np.ascontiguousarray(out_g_all.transpose(0, 2, 1))          # [48,64,N]
    f2T = np.empty((B, 2 * C, N), np.float32)
    for r in range(BH):
        b, h = divmod(r, H)
        f2T[b, h * 64:(h + 1) * 64, mf[r]] = out_gT[r].T
    wp2 = np.ascontiguousarray(w_proj.T)                                  # [768,384]
    b_eff = b_proj + w_proj[:, :] @ np.concatenate(
        [b_qkv[h * 96 + 32: h * 96 + 96] for h in range(H)])
    in_maps_b = []
    for c in range(8):
        b, half = divmod(c, 2)
        tsl = slice(half * TPB, (half + 1) * TPB)
        in_maps_b.append({
            "f2T": f2T[b][:, tsl].astype(np_b, copy=False),
            "wp2": wp2.astype(np_b, copy=False),
            "bias": b_eff.reshape(384, 1).astype(np.float32, copy=False),
        })
    t_prep2 = time.time()

    nc_b = _get("b", build_phase_b)
    t_run_b0 = time.time()
    res_b = run_bass_kernel_spmd(nc_b, in_maps_b, core_ids=list(range(8)))
    t_run_b1 = time.time()

    out = np.empty((B, N, C), np.float32)
    for c in range(8):
        b, half = divmod(c, 2)
        out[b, half * TPB:(half + 1) * TPB, :] = res_b.results[c]["outT"].T
    if os.environ.get("KTIME"):
        print(f"[kernel] prep1={t_prep1-t_prep0:.2f}s runA={t_run_a1-t_run_a0:.2f}s "
              f"prep2={t_prep2-t_run_a1:.2f}s runB={t_run_b1-t_run_b0:.2f}s")
    return out



# revision 27
# speedup vs baseline: 4.1421x; 4.1421x over previous
"""ClusterAttention Trainium2 kernel — 3-phase design.

Phase P (proj): token-order qkv projection, shared across heads.
  Each core handles (b, token-half): qkv[1152, 4096] = W^T-chunks.T @ featT,
  o-major, bf16 in/out, fp32 psum. Host pre-scales Wq rows by softmax scale.
Host gather: per (b,h) row, gather q/k/v columns into cluster order, build
  augmented q/k (20 rows: 16 qk dims + bias/pos-bias fold rows) and t-major
  v with a ones column (softmax denominator via matmul).
Phase A (attention): per core 6 rows; per cluster S'=k_aug.T@q_aug ->
  exp on ACT (psum->sbuf bf16) -> transposed AV: O[c,i] = sum_j v_t[j,c]E[j,i]
  with 256-wide moving dim; row 64 of O = denominator. Out o-major, bf16,
  unnormalized (host divides by denominator).
Host scatter: normalize, scatter to token order, build feat2T per (b, half).
Phase B (proj): outT[384, 4096] = w_proj chunks.T @ feat2T + bias.

All matmuls bf16 (1 cycle/col vs 4 for fp32); psum->sbuf copies spread over
DVE/ACT/Pool; DMAs are few and wide (>=512B contiguous runs).
"""
import os
import numpy as np
import ml_dtypes

import concourse.bacc as bacc
import concourse.tile as tile
from concourse import mybir
from concourse.bass_utils import run_bass_kernel_spmd

B, N, C, H, D, K, M = 4, 8192, 384, 12, 2, 32, 256
CH = C // H // 2            # 16
BH = B * H                  # 48
R = BH // 8                 # 6 rows (heads) per core in phase A
SCALE = float((C // H) ** -0.5)
TPB = N * B // 8            # 4096 tokens per core in phases P and B

F32 = mybir.dt.float32
BF16 = mybir.dt.bfloat16
NPBF = ml_dtypes.bfloat16
EXP = mybir.ActivationFunctionType.Exp
# copy-engine set: d=DVE, s=ACT(scalar), g=Pool(gpsimd, breaks device lowering);
# round-robined in order
KCOPY = os.environ.get("KCOPY", "ds")
BOUT = os.environ.get("BOUT", "bf16")  # phase-B output dtype


def _copy_psum(nc, dst, src, i, eng=None):
    eng = eng or KCOPY
    c = eng[i % len(eng)]
    if c == "d":
        nc.vector.tensor_copy(dst, src)
    elif c == "s":
        nc.scalar.copy(dst, src)
    else:
        nc.gpsimd.tensor_copy(dst, src)


def build_phase_p():
    nc = bacc.Bacc(None, target_bir_lowering=False)
    ft = nc.dram_tensor("ft", [3 * 128, TPB], BF16, kind="ExternalInput")
    wt = nc.dram_tensor("wt", [3 * 128, 9 * 128], BF16, kind="ExternalInput")
    qkv = nc.dram_tensor("qkv", [9 * 128, TPB], BF16, kind="ExternalOutput")
    with tile.TileContext(nc) as tc:
        with (
            tc.tile_pool(name="sb", bufs=1) as pool,
            tc.tile_pool(name="sb_o", bufs=1) as p_o,
            tc.tile_pool(name="ps", bufs=4, space="PSUM") as ps,
        ):
            ft_sb = pool.tile([128, 3 * TPB], BF16, tag="ft")
            w_sb = pool.tile([128, 3 * 1152], BF16, tag="w")
            # fused DMAs (3 cc chunks in one 3D access pattern each)
            ftr = ft.rearrange("(c p) t -> p c t", p=128)
            ftv = ft_sb.rearrange("p (c t) -> p c t", t=TPB)
            wtr = wt.rearrange("(c p) j -> p c j", p=128)
            wtv = w_sb.rearrange("p (c j) -> p c j", j=1152)
            nc.sync.dma_start(wtv[:, :, :], wtr[:, :, :])
            NT_ = TPB // 512
            for tq in range(NT_):
                q0 = tq * 512
                nc.sync.dma_start(ftv[:, :, q0:q0 + 512], ftr[:, :, q0:q0 + 512])
            o_sbs = [p_o.tile([128, TPB], BF16, tag=f"o{oc}", name=f"o{oc}")
                     for oc in range(9)]
            for tt in range(NT_):
                t0 = tt * 512
                for oc in range(9):
                    p = ps.tile([128, 512], F32, tag="p")
                    for cc in range(3):
                        nc.tensor.matmul(
                            p[:, :],
                            w_sb[:, cc * 1152 + oc * 128: cc * 1152 + (oc + 1) * 128],
                            ft_sb[:, cc * TPB + t0: cc * TPB + t0 + 512],
                            start=(cc == 0), stop=(cc == 2))
                    _copy_psum(nc, o_sbs[oc][:, t0:t0 + 512], p[:, :], oc)
                if tt % 2 == 1:
                    c0 = (tt - 1) * 512
                    for oc in range(9):
                        nc.sync.dma_start(
                            qkv[oc * 128:(oc + 1) * 128, c0:c0 + 1024],
                            o_sbs[oc][:, c0:c0 + 1024])
    nc.compile()
    return nc


def build_phase_a():
    nc = bacc.Bacc(None, target_bir_lowering=False)
    qk = nc.dram_tensor("qk", [R * 40, N], BF16, kind="ExternalInput")
    vt = nc.dram_tensor("vt", [R * 128, 64 * 65], BF16, kind="ExternalInput")
    og = nc.dram_tensor("og", [R * 65, N], BF16, kind="ExternalOutput")
    with tile.TileContext(nc) as tc:
        with (
            tc.tile_pool(name="row", bufs=2) as p_row,
            tc.tile_pool(name="e", bufs=3) as p_e,
            tc.tile_pool(name="ps_s", bufs=3, space="PSUM") as ps_s,
            tc.tile_pool(name="ps_o", bufs=2, space="PSUM") as ps_o,
        ):
            NP_ = K // 2  # cluster pairs per row
            rows = {}

            def load_row(r):
                q_sb = p_row.tile([20, N], BF16, tag="q")
                k_sb = p_row.tile([20, N], BF16, tag="k")
                v_sb = p_row.tile([128, 64 * 65], BF16, tag="v")
                o_sb = p_row.tile([65, N], BF16, tag="o")
                # chunked so the first clusters' operands land early
                nc.sync.dma_start(q_sb[:, 0:1024], qk[r * 40: r * 40 + 20, 0:1024])
                nc.sync.dma_start(k_sb[:, 0:1024],
                                  qk[r * 40 + 20: r * 40 + 40, 0:1024])
                nc.sync.dma_start(q_sb[:, 1024:], qk[r * 40: r * 40 + 20, 1024:])
                nc.sync.dma_start(k_sb[:, 1024:], qk[r * 40 + 20: r * 40 + 40, 1024:])
                nc.sync.dma_start(v_sb[:, 0: 8 * 65],
                                  vt[r * 128:(r + 1) * 128, 0: 8 * 65])
                nc.sync.dma_start(v_sb[:, 8 * 65:],
                                  vt[r * 128:(r + 1) * 128, 8 * 65:])
                rows[r] = (q_sb, k_sb, v_sb.rearrange("p (c w) -> p c w", w=65), o_sb)

            # flat software pipeline over all (row, pair) items: QK+exp runs
            # DEPTH items ahead of AV+copy so ACT never starves; input DMAs
            # prefetch one row ahead (issued on SP; out DMAs go via DVE queue
            # to keep SP's in-order stream from serializing row turnover).
            # S/E tiles span a cluster pair ([128, 1024], 2 psum banks) to
            # halve the per-activation init overhead on ACT.
            DEPTH = 2
            items = [(r, pp) for r in range(R) for pp in range(NP_)]
            queue = []
            load_row(0)
            for idx in range(len(items) + DEPTH):
                if idx < len(items):
                    r, pp = items[idx]
                    if pp == 0 and r + 1 < R:
                        load_row(r + 1)
                    q_sb, k_sb, v_view, o_sb = rows[r]
                    ps = ps_s.tile([128, 1024], F32, tag="s")
                    for u in range(2):
                        col = (pp * 2 + u) * 256
                        nc.tensor.matmul(ps[:, u * 512: u * 512 + 256],
                                         k_sb[:, col:col + 128],
                                         q_sb[:, col:col + 256],
                                         start=True, stop=True)
                        nc.tensor.matmul(ps[:, u * 512 + 256: u * 512 + 512],
                                         k_sb[:, col + 128:col + 256],
                                         q_sb[:, col:col + 256],
                                         start=True, stop=True)
                    e = p_e.tile([128, 1024], BF16, tag="e")
                    nc.scalar.activation(e[:, :], ps[:, :], EXP)
                    queue.append((r, pp, e))
                if len(queue) > DEPTH or (idx >= len(items) and queue):
                    qr, qp, qe = queue.pop(0)
                    _, _, v_view, o_sb = rows[qr]
                    po = ps_o.tile([128, 512], F32, tag="po")
                    for u in range(2):
                        for jc in range(2):
                            nc.tensor.matmul(
                                po[0:65, u * 256:(u + 1) * 256],
                                v_view[:, (qp * 2 + u) * 2 + jc, 0:65],
                                qe[:, u * 512 + jc * 256: u * 512 + (jc + 1) * 256],
                                start=(jc == 0), stop=(jc == 1))
                    _copy_psum(nc, o_sb[:, qp * 512:(qp + 1) * 512],
                               po[0:65, :], 0, eng="d")
                    # stream the row's output out in quarters to shrink the tail
                    if qp % 4 == 3:
                        c0 = (qp - 3) * 512
                        nc.gpsimd.dma_start(og[qr * 65:(qr + 1) * 65, c0:c0 + 2048],
                                            o_sb[:, c0:c0 + 2048])
    nc.compile()
    return nc


def build_phase_b():
    dt_out = F32 if BOUT == "f32" else BF16
    nc = bacc.Bacc(None, target_bir_lowering=False)
    f2T = nc.dram_tensor("f2T", [6 * 128, TPB], BF16, kind="ExternalInput")
    wp2 = nc.dram_tensor("wp2", [6 * 128, 384], BF16, kind="ExternalInput")
    bias = nc.dram_tensor("bias", [3 * 128, 1], F32, kind="ExternalInput")
    outT = nc.dram_tensor("outT", [3 * 128, TPB], dt_out, kind="ExternalOutput")
    with tile.TileContext(nc) as tc:
        with (
            tc.tile_pool(name="sb", bufs=1) as pool,
            tc.tile_pool(name="sb_o", bufs=1) as p_o,
            tc.tile_pool(name="ps", bufs=4, space="PSUM") as ps,
        ):
            fsb = pool.tile([128, 6 * TPB], BF16, tag="fsb")
            wsb = pool.tile([128, 6 * 384], BF16, tag="wsb")
            bsb = pool.tile([128, 3], F32, tag="bsb")
            # fused DMAs (6 cc chunks in one 3D access pattern each)
            nc.sync.dma_start(
                bsb.rearrange("p (c j) -> p c j", j=1)[:, :, :],
                bias.rearrange("(c p) j -> p c j", p=128)[:, :, :])
            fr = f2T.rearrange("(c p) t -> p c t", p=128)
            fv = fsb.rearrange("p (c t) -> p c t", t=TPB)
            wr = wp2.rearrange("(c p) j -> p c j", p=128)
            wv = wsb.rearrange("p (c j) -> p c j", j=384)
            nc.sync.dma_start(wv[:, :, :], wr[:, :, :])
            NT_ = TPB // 512
            for tq in range(NT_):
                q0 = tq * 512
                nc.sync.dma_start(fv[:, :, q0:q0 + 512], fr[:, :, q0:q0 + 512])
            o_sbs = [p_o.tile([128, TPB], dt_out, tag=f"o{oc}", name=f"o{oc}")
                     for oc in range(3)]
            for tt in range(NT_):
                t0 = tt * 512
                for oc in range(3):
                    p = ps.tile([128, 512], F32, tag="p")
                    for cc in range(6):
                        nc.tensor.matmul(
                            p[:, :],
                            wsb[:, cc * 384 + oc * 128: cc * 384 + (oc + 1) * 128],
                            fsb[:, cc * TPB + t0: cc * TPB + t0 + 512],
                            start=(cc == 0), stop=(cc == 5))
                    nc.vector.tensor_scalar(o_sbs[oc][:, t0:t0 + 512], p[:, :],
                                            bsb[:, oc:oc + 1], None,
                                            mybir.AluOpType.add)
                if tt % 2 == 1:
                    c0 = (tt - 1) * 512
                    for oc in range(3):
                        nc.sync.dma_start(
                            outT[oc * 128:(oc + 1) * 128, c0:c0 + 1024],
                            o_sbs[oc][:, c0:c0 + 1024])
    nc.compile()
    return nc


_CACHE = {}
PHASES = ("p", "a", "b")
_BUILDERS = {"p": build_phase_p, "a": build_phase_a, "b": build_phase_b}


def _get(name):
    if name not in _CACHE:
        _CACHE[name] = _BUILDERS[name]()
    return _CACHE[name]


def kernel(pos, feat, member_idx, w_qkv, b_qkv, w_pos, b_pos, w_proj, b_proj):
    import time
    pos = np.asarray(pos, np.float32)
    feat = np.asarray(feat, np.float32)
    mf = np.asarray(member_idx).astype(np.int64).reshape(BH, N)
    w_qkv = np.asarray(w_qkv, np.float32); b_qkv = np.asarray(b_qkv, np.float32)
    w_pos = np.asarray(w_pos, np.float32); b_pos = np.asarray(b_pos, np.float32)
    w_proj = np.asarray(w_proj, np.float32); b_proj = np.asarray(b_proj, np.float32)

    t0 = time.time()
    # ---- phase P host prep: featT per (b, half), prescaled w_qkv^T
    featT = np.ascontiguousarray(feat.transpose(0, 2, 1)).astype(NPBF)  # [B,C,N]
    w_s = w_qkv.copy()
    for h in range(H):
        w_s[h * 96: h * 96 + 16] *= SCALE          # fold softmax scale into Wq
    wt = np.ascontiguousarray(w_s.T).astype(NPBF)  # [384, 1152]
    in_p = []
    for c in range(8):
        b, half = divmod(c, 2)
        in_p.append({"ft": np.ascontiguousarray(featT[b][:, half * TPB:(half + 1) * TPB]),
                     "wt": wt})
    t1 = time.time()
    res_p = run_bass_kernel_spmd(_get("p"), in_p, core_ids=list(range(8)))
    t2 = time.time()

    # ---- host gather into cluster order + augmented rows
    qkv_all = [np.concatenate([res_p.results[2 * b]["qkv"],
                               res_p.results[2 * b + 1]["qkv"]], axis=1)
               for b in range(B)]                  # [1152, N] bf16 each
    pos_n = pos / pos.reshape(-1, D).max(0)
    b_of = np.repeat(np.arange(B), H)
    pos_g = np.take_along_axis(pos_n[b_of], mf[:, :, None], axis=1)   # [48,N,2]
    s_g = np.einsum('rnd,rd->rn', pos_g, np.tile(w_pos, (B, 1))).astype(np.float32)

    ones = np.ones((N,), NPBF)
    zeros = np.zeros((N,), NPBF)
    has_bias = bool(np.any(b_qkv))
    qk_host = np.empty((8, R * 40, N), NPBF)
    vt_host = np.empty((8, R * 128, 64 * 65), NPBF)
    for r in range(BH):
        b, h = divmod(r, H)
        core, rr = divmod(r, R)
        blk = qkv_all[b]
        idx = mf[r]
        qg = blk[h * 96: h * 96 + 16][:, idx]
        kg = blk[h * 96 + 16: h * 96 + 32][:, idx]
        vg = blk[h * 96 + 32: h * 96 + 96][:, idx]          # [64, N] bf16
        row2 = (-s_g[r]).astype(NPBF)
        row5 = (s_g[r] + b_pos[h]).astype(NPBF)
        rowqA, rowkB = zeros, zeros
        if has_bias:
            bq = b_qkv[h * 96: h * 96 + 16]
            bk = b_qkv[h * 96 + 16: h * 96 + 32]
            # qg already carries SCALE, so bk @ qg == scale*(bk . q_raw)
            rowqA = (bk @ qg.astype(np.float32)).astype(NPBF)
            row5 = (s_g[r] + b_pos[h] + SCALE * (bq @ kg.astype(np.float32))
                    + SCALE * float(bq @ bk)).astype(NPBF)
        qa = qk_host[core, rr * 40: rr * 40 + 20]
        qa[0:16] = qg; qa[16] = rowqA; qa[17] = ones; qa[18] = row2; qa[19] = ones
        ka = qk_host[core, rr * 40 + 20: rr * 40 + 40]
        ka[0:16] = kg; ka[16] = ones; ka[17] = row5; ka[18] = ones; ka[19] = rowkB
        vt = np.empty((N, 65), NPBF)
        vt[:, 0:64] = vg.T
        if has_bias:
            bv = np.concatenate([b_qkv[h * 96 + 32: h * 96 + 96]])
            vt[:, 0:64] = (vt[:, 0:64].astype(np.float32) + bv).astype(NPBF)
        vt[:, 64] = 1.0
        vt_host[core, rr * 128:(rr + 1) * 128] = (
            vt.reshape(64, 128, 65).transpose(1, 0, 2).reshape(128, 64 * 65))
    in_a = [{"qk": qk_host[c], "vt": vt_host[c]} for c in range(8)]
    t3 = time.time()
    res_a = run_bass_kernel_spmd(_get("a"), in_a, core_ids=list(range(8)))
    t4 = time.time()

    # ---- host: normalize, scatter to token order, build f2T
    f2T = np.empty((B, 2 * C, N), NPBF)
    for r in range(BH):
        b, h = divmod(r, H)
        core, rr = divmod(r, R)
        o = res_a.results[core]["og"][rr * 65:(rr + 1) * 65].astype(np.float32)
        on = o[0:64] / o[64:65]
        f2T[b][h * 64:(h + 1) * 64][:, mf[r]] = on.astype(NPBF)
    wp2 = np.ascontiguousarray(w_proj.T).astype(NPBF)       # [768, 384]
    b_eff = b_proj + w_proj @ np.concatenate(
        [b_qkv[h * 96 + 32: h * 96 + 96] for h in range(H)])
    in_b = []
    for c in range(8):
        b, half = divmod(c, 2)
        in_b.append({
            "f2T": np.ascontiguousarray(f2T[b][:, half * TPB:(half + 1) * TPB]),
            "wp2": wp2,
            "bias": b_eff.reshape(384, 1).astype(np.float32),
        })
    t5 = time.time()
    res_b = run_bass_kernel_spmd(_get("b"), in_b, core_ids=list(range(8)))
    t6 = time.time()

    out = np.empty((B, N, C), np.float32)
    for c in range(8):
        b, half = divmod(c, 2)
        out[b, half * TPB:(half + 1) * TPB, :] = \
            res_b.results[c]["outT"].astype(np.float32).T
    if os.environ.get("KTIME"):
        print(f"[kernel] prep1={t1-t0:.2f}s runP={t2-t1:.2f}s prep2={t3-t2:.2f}s "
              f"runA={t4-t3:.2f}s prep3={t5-t4:.2f}s runB={t6-t5:.2f}s")
    return out


# revision 45
# speedup vs baseline: 4.4812x; 1.0819x over previous
"""ClusterAttention Trainium2 kernel — 3-phase design.

Phase P (proj): token-order qkv projection, shared across heads.
  Each core handles (b, token-half): qkv[1152, 4096] = W^T-chunks.T @ featT,
  o-major, bf16 in/out, fp32 psum. Host pre-scales Wq rows by softmax scale.
Host gather: per (b,h) row, gather q/k/v columns into cluster order, build
  augmented q/k (20 rows: 16 qk dims + bias/pos-bias fold rows) and t-major
  v with a ones column (softmax denominator via matmul).
Phase A (attention): per core 6 rows; per cluster S'=k_aug.T@q_aug ->
  exp on ACT (psum->sbuf bf16) -> transposed AV: O[c,i] = sum_j v_t[j,c]E[j,i]
  with 256-wide moving dim; row 64 of O = denominator. Out o-major, bf16,
  unnormalized (host divides by denominator).
Host scatter: normalize, scatter to token order, build feat2T per (b, half).
Phase B (proj): outT[384, 4096] = w_proj chunks.T @ feat2T + bias.

All matmuls bf16 (1 cycle/col vs 4 for fp32); psum->sbuf copies spread over
DVE/ACT/Pool; DMAs are few and wide (>=512B contiguous runs).
"""
import os
import numpy as np
import ml_dtypes

import concourse.bacc as bacc
import concourse.tile as tile
from concourse import mybir
from concourse.bass_utils import run_bass_kernel_spmd

B, N, C, H, D, K, M = 4, 8192, 384, 12, 2, 32, 256
CH = C // H // 2            # 16
BH = B * H                  # 48
R = BH // 8                 # 6 rows (heads) per core in phase A
SCALE = float((C // H) ** -0.5)
TPB = N * B // 8            # 4096 tokens per core in phases P and B

F32 = mybir.dt.float32
BF16 = mybir.dt.bfloat16
NPBF = ml_dtypes.bfloat16
EXP = mybir.ActivationFunctionType.Exp
# copy-engine set: d=DVE, s=ACT(scalar), g=Pool(gpsimd, breaks device lowering);
# round-robined in order
KCOPY = os.environ.get("KCOPY", "ds")
BOUT = os.environ.get("BOUT", "bf16")  # phase-B output dtype


def _copy_psum(nc, dst, src, i, eng=None):
    eng = eng or KCOPY
    c = eng[i % len(eng)]
    if c == "d":
        nc.vector.tensor_copy(dst, src)
    elif c == "s":
        nc.scalar.copy(dst, src)
    else:
        nc.gpsimd.tensor_copy(dst, src)


def build_phase_p():
    nc = bacc.Bacc(None, target_bir_lowering=False)
    ft = nc.dram_tensor("ft", [3 * 128, TPB], BF16, kind="ExternalInput")
    wt = nc.dram_tensor("wt", [3 * 128, 9 * 128], BF16, kind="ExternalInput")
    qkv = nc.dram_tensor("qkv", [9 * 128, TPB], BF16, kind="ExternalOutput")
    with tile.TileContext(nc) as tc:
        with (
            tc.tile_pool(name="sb", bufs=1) as pool,
            tc.tile_pool(name="sb_o", bufs=1) as p_o,
            tc.tile_pool(name="ps", bufs=4, space="PSUM") as ps,
        ):
            ft_sb = pool.tile([128, 3 * TPB], BF16, tag="ft")
            w_sb = pool.tile([128, 3 * 1152], BF16, tag="w")
            # fused DMAs (3 cc chunks in one 3D access pattern each)
            ftr = ft.rearrange("(c p) t -> p c t", p=128)
            ftv = ft_sb.rearrange("p (c t) -> p c t", t=TPB)
            wtr = wt.rearrange("(c p) j -> p c j", p=128)
            wtv = w_sb.rearrange("p (c j) -> p c j", j=1152)
            # oc=0 weight slice first so the first matmul isn't gated on all of w
            nc.sync.dma_start(wtv[:, :, 0:128], wtr[:, :, 0:128])
            nc.sync.dma_start(ftv[:, :, 0:256], ftr[:, :, 0:256])
            nc.sync.dma_start(wtv[:, :, 128:640], wtr[:, :, 128:640])
            nc.sync.dma_start(ftv[:, :, 256:512], ftr[:, :, 256:512])
            nc.sync.dma_start(wtv[:, :, 640:], wtr[:, :, 640:])
            NT_ = TPB // 512
            for tq in range(1, NT_):
                q0 = tq * 512
                nc.sync.dma_start(ftv[:, :, q0:q0 + 512], ftr[:, :, q0:q0 + 512])
            o_all = pool.tile([128, 9 * TPB], BF16, tag="o_all")
            o_v = o_all.rearrange("p (c t) -> p c t", t=TPB)
            qkv_v = qkv.rearrange("(c p) t -> p c t", p=128)
            # token tiles taper at the end so the final drain DMA is tiny;
            # fused 3D out DMAs (all 9 oc at once) keep the HWDGE count low
            widths = [512] * 7 + [256, 256]
            drains = [(0, 1024), (1024, 1024), (2048, 1024), (3072, 512),
                      (3584, 256), (3840, 256)]
            t0 = 0
            di = 0
            for tt, w in enumerate(widths):
                for oc in range(9):
                    p = ps.tile([128, 512], F32, tag="p")
                    for cc in range(3):
                        nc.tensor.matmul(
                            p[:, 0:w],
                            w_sb[:, cc * 1152 + oc * 128: cc * 1152 + (oc + 1) * 128],
                            ft_sb[:, cc * TPB + t0: cc * TPB + t0 + w],
                            start=(cc == 0), stop=(cc == 2))
                    _copy_psum(nc, o_v[:, oc, t0:t0 + w], p[:, 0:w], oc)
                t0 += w
                while di < len(drains) and drains[di][0] + drains[di][1] <= t0:
                    d0, dw = drains[di]
                    di += 1
                    nc.sync.dma_start(qkv_v[:, :, d0:d0 + dw],
                                      o_v[:, :, d0:d0 + dw])
    nc.compile()
    return nc


def build_phase_a():
    nc = bacc.Bacc(None, target_bir_lowering=False)
    qk = nc.dram_tensor("qk", [R * 40, N], BF16, kind="ExternalInput")
    vt = nc.dram_tensor("vt", [R * 128, 64 * 65], BF16, kind="ExternalInput")
    og = nc.dram_tensor("og", [R * 65, N], BF16, kind="ExternalOutput")
    with tile.TileContext(nc) as tc:
        with (
            tc.tile_pool(name="row", bufs=2) as p_row,
            tc.tile_pool(name="e", bufs=int(os.environ.get("KEB", "5"))) as p_e,
            tc.tile_pool(name="ps_s", bufs=(2 if os.environ.get("KGRP", "3") == "3" else 3), space="PSUM") as ps_s,
            tc.tile_pool(name="ps_o", bufs=2, space="PSUM") as ps_o,
        ):
            NP_ = K // 2  # cluster pairs per row
            rows = {}

            # warm the ACT exp table at t=0 so the 1.3us load hides under DMA,
            # and run dummy matmuls so the PE p-state ramps while DMAs fly
            scratch = p_e.tile([1, 8], F32, tag="warm")
            nc.vector.memset(scratch[:, :], 0.0)
            nc.scalar.activation(scratch[:, 4:8], scratch[:, 0:4], EXP)
            wsb = p_e.tile([20, 256], BF16, tag="wsb")
            nc.vector.memset(wsb[:, :], 0.0)
            wps = ps_o.tile([128, 512], F32, tag="po")
            for i in range(14):
                nc.tensor.matmul(wps[:, 0:256], wsb[:, 0:128], wsb[:, 0:256],
                                 start=True, stop=True)

            def load_row(r):
                q_sb = p_row.tile([20, N], BF16, tag="q")
                k_sb = p_row.tile([20, N], BF16, tag="k")
                v_sb = p_row.tile([128, 64 * 65], BF16, tag="v")
                o_sb = p_row.tile([65, N], BF16, tag="o")
                # chunked so the first clusters' operands land early
                for lo, hi in ((0, 256), (256, 2048), (2048, N)):
                    nc.sync.dma_start(q_sb[:, lo:hi], qk[r * 40: r * 40 + 20, lo:hi])
                    nc.sync.dma_start(k_sb[:, lo:hi],
                                      qk[r * 40 + 20: r * 40 + 40, lo:hi])
                nc.sync.dma_start(v_sb[:, 0: 8 * 65],
                                  vt[r * 128:(r + 1) * 128, 0: 8 * 65])
                nc.sync.dma_start(v_sb[:, 8 * 65:],
                                  vt[r * 128:(r + 1) * 128, 8 * 65:])
                rows[r] = (q_sb, k_sb, v_sb.rearrange("p (c w) -> p c w", w=65), o_sb)

            # Flat software pipeline over all clusters: QK+exp runs DEPTH_C
            # clusters ahead of AV+copy so ACT (the bottleneck) never starves.
            # S/E tiles batch 3 clusters ([128, 1536], 3 psum banks) to cut the
            # per-activation init overhead; AV works in 2-cluster units.
            # Input DMAs prefetch one row ahead on SP; out DMAs stream via the
            # idle Pool SWDGE queue, except each row's last two chunks which
            # ride SP/inline so the program tail is one small hop.
            DEPTH_C = int(os.environ.get("KDC", "7"))
            GROUPS = ([3] * 10 + [2]) if os.environ.get("KGRP", "3") == "3" else [2] * 16
            sitems = []
            for r in range(R):
                c0 = 0
                for g in GROUPS:
                    sitems.append((r, c0, g))
                    c0 += g
            e_of = {}
            av_list = [(r, pp) for r in range(R) for pp in range(NP_)]
            av_ptr = 0
            issued = 0
            done = 0

            def do_av():
                nonlocal av_ptr, done
                qr, qp = av_list[av_ptr]
                av_ptr += 1
                done += 2
                _, _, v_view, o_sb = rows[qr]
                po = ps_o.tile([128, 512], F32, tag="po")
                for u in range(2):
                    et, off = e_of.pop((qr, qp * 2 + u))
                    for jc in range(2):
                        nc.tensor.matmul(
                            po[0:65, u * 256:(u + 1) * 256],
                            v_view[:, (qp * 2 + u) * 2 + jc, 0:65],
                            et[:, off + jc * 256: off + (jc + 1) * 256],
                            start=(jc == 0), stop=(jc == 1))
                _copy_psum(nc, o_sb[:, qp * 512:(qp + 1) * 512],
                           po[0:65, :], 0, eng="d")
                # stream the row's output: big chunks via Pool SWDGE, the last
                # two pairs per-chunk on SP right behind their copies
                if qp == 7:
                    nc.gpsimd.dma_start(og[qr * 65:(qr + 1) * 65, 0:4096],
                                        o_sb[:, 0:4096])
                elif qp == 13:
                    nc.gpsimd.dma_start(og[qr * 65:(qr + 1) * 65, 4096:7168],
                                        o_sb[:, 4096:7168])
                elif qp >= 14:
                    nc.sync.dma_start(
                        og[qr * 65:(qr + 1) * 65, qp * 512:(qp + 1) * 512],
                        o_sb[:, qp * 512:(qp + 1) * 512])

            load_row(0)
            for r, c0, g in sitems:
                if c0 == 0 and r + 1 < R:
                    load_row(r + 1)
                # drain AV work first so PE has queued work while the next
                # S-group's psum recycles through the pending exp; taper the
                # lag over the last groups so the end-flush backlog is small
                lag = DEPTH_C + g
                left = len(sitems) - sitems.index((r, c0, g))
                if left <= 3:
                    lag = min(lag, 2 * left)
                while av_ptr < len(av_list) and issued - done >= lag:
                    do_av()
                q_sb, k_sb, v_view, o_sb = rows[r]
                ps = ps_s.tile([128, 512 * g], F32, tag="s",
                               padded_shape=[128, 512 * max(GROUPS)])
                for i in range(g):
                    col = (c0 + i) * 256
                    nc.tensor.matmul(ps[:, i * 512: i * 512 + 256],
                                     k_sb[:, col:col + 128],
                                     q_sb[:, col:col + 256],
                                     start=True, stop=True)
                    nc.tensor.matmul(ps[:, i * 512 + 256: i * 512 + 512],
                                     k_sb[:, col + 128:col + 256],
                                     q_sb[:, col:col + 256],
                                     start=True, stop=True)
                e = p_e.tile([128, 512 * g], BF16, tag="e",
                             padded_shape=[128, 512 * max(GROUPS)])
                nc.scalar.activation(e[:, :], ps[:, :], EXP)
                for i in range(g):
                    e_of[(r, c0 + i)] = (e, i * 512)
                issued += g
            while av_ptr < len(av_list):
                do_av()
    nc.compile()
    return nc


def build_phase_b():
    dt_out = F32 if BOUT == "f32" else BF16
    nc = bacc.Bacc(None, target_bir_lowering=False)
    f2T = nc.dram_tensor("f2T", [6 * 128, TPB], BF16, kind="ExternalInput")
    wp2 = nc.dram_tensor("wp2", [6 * 128, 384], BF16, kind="ExternalInput")
    bias = nc.dram_tensor("bias", [3 * 128, 1], F32, kind="ExternalInput")
    outT = nc.dram_tensor("outT", [3 * 128, TPB], dt_out, kind="ExternalOutput")
    with tile.TileContext(nc) as tc:
        with (
            tc.tile_pool(name="sb", bufs=1) as pool,
            tc.tile_pool(name="sb_o", bufs=1) as p_o,
            tc.tile_pool(name="ps", bufs=4, space="PSUM") as ps,
        ):
            fsb = pool.tile([128, 6 * TPB], BF16, tag="fsb")
            wsb = pool.tile([128, 6 * 384], BF16, tag="wsb")
            bsb = pool.tile([128, 3], F32, tag="bsb")
            # fused DMAs (6 cc chunks in one 3D access pattern each)
            nc.sync.dma_start(
                bsb.rearrange("p (c j) -> p c j", j=1)[:, :, :],
                bias.rearrange("(c p) j -> p c j", p=128)[:, :, :])
            fr = f2T.rearrange("(c p) t -> p c t", p=128)
            fv = fsb.rearrange("p (c t) -> p c t", t=TPB)
            wr = wp2.rearrange("(c p) j -> p c j", p=128)
            wv = wsb.rearrange("p (c j) -> p c j", j=384)
            nc.sync.dma_start(wv[:, :, 0:128], wr[:, :, 0:128])
            nc.sync.dma_start(fv[:, :, 0:256], fr[:, :, 0:256])
            nc.sync.dma_start(wv[:, :, 128:], wr[:, :, 128:])
            nc.sync.dma_start(fv[:, :, 256:512], fr[:, :, 256:512])
            NT_ = TPB // 512
            for tq in range(1, NT_):
                q0 = tq * 512
                nc.sync.dma_start(fv[:, :, q0:q0 + 512], fr[:, :, q0:q0 + 512])
            o_all = pool.tile([128, 3 * TPB], dt_out, tag="o_all")
            o_v = o_all.rearrange("p (c t) -> p c t", t=TPB)
            out_v = outT.rearrange("(c p) t -> p c t", p=128)
            widths = [512] * 7 + [256, 256]
            drains = [(0, 1024), (1024, 1024), (2048, 1024), (3072, 512),
                      (3584, 256), (3840, 256)]
            t0 = 0
            di = 0
            for tt, w in enumerate(widths):
                for oc in range(3):
                    p = ps.tile([128, 512], F32, tag="p")
                    for cc in range(6):
                        nc.tensor.matmul(
                            p[:, 0:w],
                            wsb[:, cc * 384 + oc * 128: cc * 384 + (oc + 1) * 128],
                            fsb[:, cc * TPB + t0: cc * TPB + t0 + w],
                            start=(cc == 0), stop=(cc == 5))
                    nc.vector.tensor_scalar(o_v[:, oc, t0:t0 + w], p[:, 0:w],
                                            bsb[:, oc:oc + 1], None,
                                            mybir.AluOpType.add)
                t0 += w
                while di < len(drains) and drains[di][0] + drains[di][1] <= t0:
                    d0, dw = drains[di]
                    di += 1
                    nc.sync.dma_start(out_v[:, :, d0:d0 + dw],
                                      o_v[:, :, d0:d0 + dw])
    nc.compile()
    return nc


_CACHE = {}
PHASES = ("p", "a", "b")
_BUILDERS = {"p": build_phase_p, "a": build_phase_a, "b": build_phase_b}


def _get(name):
    if name not in _CACHE:
        _CACHE[name] = _BUILDERS[name]()
    return _CACHE[name]


def kernel(pos, feat, member_idx, w_qkv, b_qkv, w_pos, b_pos, w_proj, b_proj):
    import time
    pos = np.asarray(pos, np.float32)
    feat = np.asarray(feat, np.float32)
    mf = np.asarray(member_idx).astype(np.int64).reshape(BH, N)
    w_qkv = np.asarray(w_qkv, np.float32); b_qkv = np.asarray(b_qkv, np.float32)
    w_pos = np.asarray(w_pos, np.float32); b_pos = np.asarray(b_pos, np.float32)
    w_proj = np.asarray(w_proj, np.float32); b_proj = np.asarray(b_proj, np.float32)

    t0 = time.time()
    # ---- phase P host prep: featT per (b, half), prescaled w_qkv^T
    featT = np.ascontiguousarray(feat.transpose(0, 2, 1)).astype(NPBF)  # [B,C,N]
    w_s = w_qkv.copy()
    for h in range(H):
        w_s[h * 96: h * 96 + 16] *= SCALE          # fold softmax scale into Wq
    wt = np.ascontiguousarray(w_s.T).astype(NPBF)  # [384, 1152]
    in_p = []
    for c in range(8):
        b, half = divmod(c, 2)
        in_p.append({"ft": np.ascontiguousarray(featT[b][:, half * TPB:(half + 1) * TPB]),
                     "wt": wt})
    t1 = time.time()
    res_p = run_bass_kernel_spmd(_get("p"), in_p, core_ids=list(range(8)))
    t2 = time.time()

    # ---- host gather into cluster order + augmented rows
    qkv_all = [np.concatenate([res_p.results[2 * b]["qkv"],
                               res_p.results[2 * b + 1]["qkv"]], axis=1)
               for b in range(B)]                  # [1152, N] bf16 each
    pos_n = pos / pos.reshape(-1, D).max(0)
    b_of = np.repeat(np.arange(B), H)
    pos_g = np.take_along_axis(pos_n[b_of], mf[:, :, None], axis=1)   # [48,N,2]
    s_g = np.einsum('rnd,rd->rn', pos_g, np.tile(w_pos, (B, 1))).astype(np.float32)

    ones = np.ones((N,), NPBF)
    zeros = np.zeros((N,), NPBF)
    has_bias = bool(np.any(b_qkv))
    qk_host = np.empty((8, R * 40, N), NPBF)
    vt_host = np.empty((8, R * 128, 64 * 65), NPBF)
    for r in range(BH):
        b, h = divmod(r, H)
        core, rr = divmod(r, R)
        blk = qkv_all[b]
        idx = mf[r]
        qg = blk[h * 96: h * 96 + 16][:, idx]
        kg = blk[h * 96 + 16: h * 96 + 32][:, idx]
        vg = blk[h * 96 + 32: h * 96 + 96][:, idx]          # [64, N] bf16
        row2 = (-s_g[r]).astype(NPBF)
        row5 = (s_g[r] + b_pos[h]).astype(NPBF)
        rowqA, rowkB = zeros, zeros
        if has_bias:
            bq = b_qkv[h * 96: h * 96 + 16]
            bk = b_qkv[h * 96 + 16: h * 96 + 32]
            # qg already carries SCALE, so bk @ qg == scale*(bk . q_raw)
            rowqA = (bk @ qg.astype(np.float32)).astype(NPBF)
            row5 = (s_g[r] + b_pos[h] + SCALE * (bq @ kg.astype(np.float32))
                    + SCALE * float(bq @ bk)).astype(NPBF)
        qa = qk_host[core, rr * 40: rr * 40 + 20]
        qa[0:16] = qg; qa[16] = rowqA; qa[17] = ones; qa[18] = row2; qa[19] = ones
        ka = qk_host[core, rr * 40 + 20: rr * 40 + 40]
        ka[0:16] = kg; ka[16] = ones; ka[17] = row5; ka[18] = ones; ka[19] = rowkB
        vt = np.empty((N, 65), NPBF)
        vt[:, 0:64] = vg.T
        if has_bias:
            bv = np.concatenate([b_qkv[h * 96 + 32: h * 96 + 96]])
            vt[:, 0:64] = (vt[:, 0:64].astype(np.float32) + bv).astype(NPBF)
        vt[:, 64] = 1.0
        vt_host[core, rr * 128:(rr + 1) * 128] = (
            vt.reshape(64, 128, 65).transpose(1, 0, 2).reshape(128, 64 * 65))
    in_a = [{"qk": qk_host[c], "vt": vt_host[c]} for c in range(8)]
    t3 = time.time()
    res_a = run_bass_kernel_spmd(_get("a"), in_a, core_ids=list(range(8)))
    t4 = time.time()

    # ---- host: normalize, scatter to token order, build f2T
    f2T = np.empty((B, 2 * C, N), NPBF)
    for r in range(BH):
        b, h = divmod(r, H)
        core, rr = divmod(r, R)
        o = res_a.results[core]["og"][rr * 65:(rr + 1) * 65].astype(np.float32)
        on = o[0:64] / o[64:65]
        f2T[b][h * 64:(h + 1) * 64][:, mf[r]] = on.astype(NPBF)
    wp2 = np.ascontiguousarray(w_proj.T).astype(NPBF)       # [768, 384]
    b_eff = b_proj + w_proj @ np.concatenate(
        [b_qkv[h * 96 + 32: h * 96 + 96] for h in range(H)])
    in_b = []
    for c in range(8):
        b, half = divmod(c, 2)
        in_b.append({
            "f2T": np.ascontiguousarray(f2T[b][:, half * TPB:(half + 1) * TPB]),
            "wp2": wp2,
            "bias": b_eff.reshape(384, 1).astype(np.float32),
        })
    t5 = time.time()
    res_b = run_bass_kernel_spmd(_get("b"), in_b, core_ids=list(range(8)))
    t6 = time.time()

    out = np.empty((B, N, C), np.float32)
    for c in range(8):
        b, half = divmod(c, 2)
        out[b, half * TPB:(half + 1) * TPB, :] = \
            res_b.results[c]["outT"].astype(np.float32).T
    if os.environ.get("KTIME"):
        print(f"[kernel] prep1={t1-t0:.2f}s runP={t2-t1:.2f}s prep2={t3-t2:.2f}s "
              f"runA={t4-t3:.2f}s prep3={t5-t4:.2f}s runB={t6-t5:.2f}s")
    return out


# revision 51
# speedup vs baseline: 4.6314x; 1.0335x over previous
"""ClusterAttention Trainium2 kernel — 3-phase design.

Phase P (proj): token-order qkv projection, shared across heads.
  Each core handles (b, token-half): qkv[1152, 4096] = W^T-chunks.T @ featT,
  o-major, bf16 in/out, fp32 psum. Host pre-scales Wq rows by softmax scale.
Host gather: per (b,h) row, gather q/k/v columns into cluster order, build
  augmented q/k (20 rows: 16 qk dims + bias/pos-bias fold rows) and t-major
  v with a ones column (softmax denominator via matmul).
Phase A (attention): per core 6 rows; per cluster S'=k_aug.T@q_aug ->
  exp on ACT (psum->sbuf bf16) -> transposed AV: O[c,i] = sum_j v_t[j,c]E[j,i]
  with 256-wide moving dim; row 64 of O = denominator. Out o-major, bf16,
  unnormalized (host divides by denominator).
Host scatter: normalize, scatter to token order, build feat2T per (b, half).
Phase B (proj): outT[384, 4096] = w_proj chunks.T @ feat2T + bias.

All matmuls bf16 (1 cycle/col vs 4 for fp32); psum->sbuf copies spread over
DVE/ACT/Pool; DMAs are few and wide (>=512B contiguous runs).
"""
import os
import numpy as np
import ml_dtypes

import concourse.bacc as bacc
import concourse.tile as tile
from concourse import mybir
from concourse.bass_utils import run_bass_kernel_spmd

B, N, C, H, D, K, M = 4, 8192, 384, 12, 2, 32, 256
CH = C // H // 2            # 16
BH = B * H                  # 48
R = BH // 8                 # 6 rows (heads) per core in phase A
SCALE = float((C // H) ** -0.5)
TPB = N * B // 8            # 4096 tokens per core in phases P and B

F32 = mybir.dt.float32
BF16 = mybir.dt.bfloat16
NPBF = ml_dtypes.bfloat16
EXP = mybir.ActivationFunctionType.Exp
# copy-engine set: d=DVE, s=ACT(scalar), g=Pool(gpsimd, breaks device lowering);
# round-robined in order
KCOPY = os.environ.get("KCOPY", "ds")
BOUT = os.environ.get("BOUT", "bf16")  # phase-B output dtype


def _copy_psum(nc, dst, src, i, eng=None):
    eng = eng or KCOPY
    c = eng[i % len(eng)]
    if c == "d":
        nc.vector.tensor_copy(dst, src)
    elif c == "s":
        nc.scalar.copy(dst, src)
    else:
        nc.gpsimd.tensor_copy(dst, src)


def build_phase_p():
    nc = bacc.Bacc(None, target_bir_lowering=False)
    ft = nc.dram_tensor("ft", [3 * 128, TPB], BF16, kind="ExternalInput")
    wt = nc.dram_tensor("wt", [3 * 128, 9 * 128], BF16, kind="ExternalInput")
    qkv = nc.dram_tensor("qkv", [9 * 128, TPB], BF16, kind="ExternalOutput")
    with tile.TileContext(nc) as tc:
        with (
            tc.tile_pool(name="sb", bufs=1) as pool,
            tc.tile_pool(name="sb_o", bufs=1) as p_o,
            tc.tile_pool(name="ps", bufs=4, space="PSUM") as ps,
        ):
            ft_sb = pool.tile([128, 3 * TPB], BF16, tag="ft")
            w_sb = pool.tile([128, 3 * 1152], BF16, tag="w")
            warm = pool.tile([20, 256], BF16, tag="warm")
            nc.vector.memset(warm[:, :], 0.0)
            wps = ps.tile([128, 512], F32, tag="p")
            for i in range(10):
                nc.tensor.matmul(wps[:, 0:256], warm[:, 0:128], warm[:, 0:256],
                                 start=True, stop=True)
            # fused DMAs (3 cc chunks in one 3D access pattern each)
            ftr = ft.rearrange("(c p) t -> p c t", p=128)
            ftv = ft_sb.rearrange("p (c t) -> p c t", t=TPB)
            wtr = wt.rearrange("(c p) j -> p c j", p=128)
            wtv = w_sb.rearrange("p (c j) -> p c j", j=1152)
            # oc=0 weight slice first so the first matmul isn't gated on all of w
            nc.sync.dma_start(wtv[:, :, 0:256], wtr[:, :, 0:256])
            nc.sync.dma_start(ftv[:, :, 0:512], ftr[:, :, 0:512])
            nc.sync.dma_start(wtv[:, :, 256:], wtr[:, :, 256:])
            for q0 in range(512, TPB, 512):
                nc.sync.dma_start(ftv[:, :, q0:q0 + 512], ftr[:, :, q0:q0 + 512])
            o_all = pool.tile([128, 9 * TPB], BF16, tag="o_all")
            o_v = o_all.rearrange("p (c t) -> p c t", t=TPB)
            qkv_v = qkv.rearrange("(c p) t -> p c t", p=128)
            # token tiles taper at the end so the final drain DMA is tiny;
            # fused 3D out DMAs (all 9 oc at once) keep the HWDGE count low
            widths = [512] * 7 + [256, 256]
            drains = [(0, 1024), (1024, 1024), (2048, 1024), (3072, 512),
                      (3584, 256), (3840, 256)]
            t0 = 0
            di = 0
            for tt, w in enumerate(widths):
                for oc in range(9):
                    p = ps.tile([128, 512], F32, tag="p")
                    for cc in range(3):
                        nc.tensor.matmul(
                            p[:, 0:w],
                            w_sb[:, cc * 1152 + oc * 128: cc * 1152 + (oc + 1) * 128],
                            ft_sb[:, cc * TPB + t0: cc * TPB + t0 + w],
                            start=(cc == 0), stop=(cc == 2))
                    _copy_psum(nc, o_v[:, oc, t0:t0 + w], p[:, 0:w], oc)
                t0 += w
                while di < len(drains) and drains[di][0] + drains[di][1] <= t0:
                    d0, dw = drains[di]
                    di += 1
                    nc.sync.dma_start(qkv_v[:, :, d0:d0 + dw],
                                      o_v[:, :, d0:d0 + dw])
    nc.compile()
    return nc


def build_phase_a():
    nc = bacc.Bacc(None, target_bir_lowering=False)
    qk = nc.dram_tensor("qk", [R * 40, N], BF16, kind="ExternalInput")
    vt = nc.dram_tensor("vt", [R * 128, 64 * 65], BF16, kind="ExternalInput")
    og = nc.dram_tensor("og", [R * 65, N], BF16, kind="ExternalOutput")
    with tile.TileContext(nc) as tc:
        with (
            tc.tile_pool(name="row", bufs=2) as p_row,
            tc.tile_pool(name="e", bufs=int(os.environ.get("KEB", "5"))) as p_e,
            tc.tile_pool(name="ps_s", bufs=(2 if os.environ.get("KGRP", "3") == "3" else 3), space="PSUM") as ps_s,
            tc.tile_pool(name="ps_o", bufs=2, space="PSUM") as ps_o,
        ):
            NP_ = K // 2  # cluster pairs per row
            rows = {}

            # warm the ACT exp table at t=0 so the 1.3us load hides under DMA,
            # and run dummy matmuls so the PE p-state ramps while DMAs fly
            scratch = p_e.tile([1, 8], F32, tag="warm")
            nc.vector.memset(scratch[:, :], 0.0)
            nc.scalar.activation(scratch[:, 4:8], scratch[:, 0:4], EXP)
            wsb = p_e.tile([20, 256], BF16, tag="wsb")
            nc.vector.memset(wsb[:, :], 0.0)
            wps = ps_o.tile([128, 512], F32, tag="po")
            for i in range(14):
                nc.tensor.matmul(wps[:, 0:256], wsb[:, 0:128], wsb[:, 0:256],
                                 start=True, stop=True)

            def load_row(r):
                q_sb = p_row.tile([20, N], BF16, tag="q")
                k_sb = p_row.tile([20, N], BF16, tag="k")
                v_sb = p_row.tile([128, 64 * 65], BF16, tag="v")
                o_sb = p_row.tile([65, N], BF16, tag="o")
                # chunked so the first clusters' operands land early
                for lo, hi in ((0, 256), (256, 2048), (2048, N)):
                    nc.sync.dma_start(q_sb[:, lo:hi], qk[r * 40: r * 40 + 20, lo:hi])
                    nc.sync.dma_start(k_sb[:, lo:hi],
                                      qk[r * 40 + 20: r * 40 + 40, lo:hi])
                nc.sync.dma_start(v_sb[:, 0: 8 * 65],
                                  vt[r * 128:(r + 1) * 128, 0: 8 * 65])
                nc.sync.dma_start(v_sb[:, 8 * 65:],
                                  vt[r * 128:(r + 1) * 128, 8 * 65:])
                rows[r] = (q_sb, k_sb, v_sb.rearrange("p (c w) -> p c w", w=65), o_sb)

            # Flat software pipeline over all clusters: QK+exp runs DEPTH_C
            # clusters ahead of AV+copy so ACT (the bottleneck) never starves.
            # S/E tiles batch 3 clusters ([128, 1536], 3 psum banks) to cut the
            # per-activation init overhead; AV works in 2-cluster units.
            # Input DMAs prefetch one row ahead on SP; out DMAs stream via the
            # idle Pool SWDGE queue, except each row's last two chunks which
            # ride SP/inline so the program tail is one small hop.
            DEPTH_C = int(os.environ.get("KDC", "8"))
            GROUPS = ([3] * 10 + [2]) if os.environ.get("KGRP", "3") == "3" else [2] * 16
            sitems = []
            for r in range(R):
                c0 = 0
                for g in GROUPS:
                    sitems.append((r, c0, g))
                    c0 += g
            e_of = {}
            av_list = [(r, pp) for r in range(R) for pp in range(NP_)]
            av_ptr = 0
            issued = 0
            done = 0

            def do_av(eng="d"):
                nonlocal av_ptr, done
                qr, qp = av_list[av_ptr]
                av_ptr += 1
                done += 2
                _, _, v_view, o_sb = rows[qr]
                po = ps_o.tile([128, 512], F32, tag="po")
                for u in range(2):
                    et, off = e_of.pop((qr, qp * 2 + u))
                    for jc in range(2):
                        nc.tensor.matmul(
                            po[0:65, u * 256:(u + 1) * 256],
                            v_view[:, (qp * 2 + u) * 2 + jc, 0:65],
                            et[:, off + jc * 256: off + (jc + 1) * 256],
                            start=(jc == 0), stop=(jc == 1))
                _copy_psum(nc, o_sb[:, qp * 512:(qp + 1) * 512],
                           po[0:65, :], 0, eng=eng)
                # stream the row's output: big chunks via Pool SWDGE, the last
                # two pairs per-chunk on SP right behind their copies
                if qp == 7:
                    nc.gpsimd.dma_start(og[qr * 65:(qr + 1) * 65, 0:4096],
                                        o_sb[:, 0:4096])
                elif qp == 13:
                    nc.gpsimd.dma_start(og[qr * 65:(qr + 1) * 65, 4096:7168],
                                        o_sb[:, 4096:7168])
                elif qp >= 14:
                    nc.sync.dma_start(
                        og[qr * 65:(qr + 1) * 65, qp * 512:(qp + 1) * 512],
                        o_sb[:, qp * 512:(qp + 1) * 512])

            load_row(0)
            for r, c0, g in sitems:
                if c0 == 0 and r + 1 < R:
                    load_row(r + 1)
                # drain AV work first so PE has queued work while the next
                # S-group's psum recycles through the pending exp; taper the
                # lag over the last groups so the end-flush backlog is small
                lag = DEPTH_C + g
                left = len(sitems) - sitems.index((r, c0, g))
                if left <= 3:
                    lag = min(lag, 2 * left)
                while av_ptr < len(av_list) and issued - done >= lag:
                    do_av()
                q_sb, k_sb, v_view, o_sb = rows[r]
                ps = ps_s.tile([128, 512 * g], F32, tag="s",
                               padded_shape=[128, 512 * max(GROUPS)])
                for i in range(g):
                    col = (c0 + i) * 256
                    nc.tensor.matmul(ps[:, i * 512: i * 512 + 256],
                                     k_sb[:, col:col + 128],
                                     q_sb[:, col:col + 256],
                                     start=True, stop=True)
                    nc.tensor.matmul(ps[:, i * 512 + 256: i * 512 + 512],
                                     k_sb[:, col + 128:col + 256],
                                     q_sb[:, col:col + 256],
                                     start=True, stop=True)
                e = p_e.tile([128, 512 * g], BF16, tag="e",
                             padded_shape=[128, 512 * max(GROUPS)])
                nc.scalar.activation(e[:, :], ps[:, :], EXP)
                for i in range(g):
                    e_of[(r, c0 + i)] = (e, i * 512)
                issued += g
            flush_i = 0
            while av_ptr < len(av_list):
                do_av(eng="ds"[flush_i % 2])
                flush_i += 1
    nc.compile()
    return nc


def build_phase_b():
    dt_out = F32 if BOUT == "f32" else BF16
    nc = bacc.Bacc(None, target_bir_lowering=False)
    f2T = nc.dram_tensor("f2T", [6 * 128, TPB], BF16, kind="ExternalInput")
    wp2 = nc.dram_tensor("wp2", [6 * 128, 384], BF16, kind="ExternalInput")
    bias = nc.dram_tensor("bias", [3 * 128, 1], F32, kind="ExternalInput")
    outT = nc.dram_tensor("outT", [3 * 128, TPB], dt_out, kind="ExternalOutput")
    with tile.TileContext(nc) as tc:
        with (
            tc.tile_pool(name="sb", bufs=1) as pool,
            tc.tile_pool(name="sb_o", bufs=1) as p_o,
            tc.tile_pool(name="ps", bufs=4, space="PSUM") as ps,
        ):
            fsb = pool.tile([128, 6 * TPB], BF16, tag="fsb")
            wsb = pool.tile([128, 6 * 384], BF16, tag="wsb")
            bsb = pool.tile([128, 3], F32, tag="bsb")
            warm = pool.tile([20, 256], BF16, tag="warm")
            nc.vector.memset(warm[:, :], 0.0)
            wps = ps.tile([128, 512], F32, tag="p")
            for i in range(12):
                nc.tensor.matmul(wps[:, 0:256], warm[:, 0:128], warm[:, 0:256],
                                 start=True, stop=True)
            # fused DMAs (6 cc chunks in one 3D access pattern each)
            nc.sync.dma_start(
                bsb.rearrange("p (c j) -> p c j", j=1)[:, :, :],
                bias.rearrange("(c p) j -> p c j", p=128)[:, :, :])
            fr = f2T.rearrange("(c p) t -> p c t", p=128)
            fv = fsb.rearrange("p (c t) -> p c t", t=TPB)
            wr = wp2.rearrange("(c p) j -> p c j", p=128)
            wv = wsb.rearrange("p (c j) -> p c j", j=384)
            nc.sync.dma_start(wv[:, :, 0:256], wr[:, :, 0:256])
            nc.sync.dma_start(fv[:, :, 0:256], fr[:, :, 0:256])
            nc.sync.dma_start(wv[:, :, 256:], wr[:, :, 256:])
            nc.sync.dma_start(fv[:, :, 256:512], fr[:, :, 256:512])
            for q0 in range(512, TPB, 512):
                nc.sync.dma_start(fv[:, :, q0:q0 + 512], fr[:, :, q0:q0 + 512])
            o_all = pool.tile([128, 3 * TPB], dt_out, tag="o_all")
            o_v = o_all.rearrange("p (c t) -> p c t", t=TPB)
            out_v = outT.rearrange("(c p) t -> p c t", p=128)
            widths = [256, 256] + [512] * 6 + [256, 256]
            drains = [(0, 512), (512, 1024), (1536, 1024), (2560, 1024),
                      (3584, 256), (3840, 256)]
            t0 = 0
            di = 0
            for tt, w in enumerate(widths):
                for oc in range(3):
                    p = ps.tile([128, 512], F32, tag="p")
                    for cc in range(6):
                        nc.tensor.matmul(
                            p[:, 0:w],
                            wsb[:, cc * 384 + oc * 128: cc * 384 + (oc + 1) * 128],
                            fsb[:, cc * TPB + t0: cc * TPB + t0 + w],
                            start=(cc == 0), stop=(cc == 5))
                    nc.vector.tensor_scalar(o_v[:, oc, t0:t0 + w], p[:, 0:w],
                                            bsb[:, oc:oc + 1], None,
                                            mybir.AluOpType.add)
                t0 += w
                while di < len(drains) and drains[di][0] + drains[di][1] <= t0:
                    d0, dw = drains[di]
                    di += 1
                    nc.sync.dma_start(out_v[:, :, d0:d0 + dw],
                                      o_v[:, :, d0:d0 + dw])
    nc.compile()
    return nc


_CACHE = {}
PHASES = ("p", "a", "b")
_BUILDERS = {"p": build_phase_p, "a": build_phase_a, "b": build_phase_b}


def _get(name):
    if name not in _CACHE:
        _CACHE[name] = _BUILDERS[name]()
    return _CACHE[name]


def kernel(pos, feat, member_idx, w_qkv, b_qkv, w_pos, b_pos, w_proj, b_proj):
    import time
    pos = np.asarray(pos, np.float32)
    feat = np.asarray(feat, np.float32)
    mf = np.asarray(member_idx).astype(np.int64).reshape(BH, N)
    w_qkv = np.asarray(w_qkv, np.float32); b_qkv = np.asarray(b_qkv, np.float32)
    w_pos = np.asarray(w_pos, np.float32); b_pos = np.asarray(b_pos, np.float32)
    w_proj = np.asarray(w_proj, np.float32); b_proj = np.asarray(b_proj, np.float32)

    t0 = time.time()
    # ---- phase P host prep: featT per (b, half), prescaled w_qkv^T
    featT = np.ascontiguousarray(feat.transpose(0, 2, 1)).astype(NPBF)  # [B,C,N]
    w_s = w_qkv.copy()
    for h in range(H):
        w_s[h * 96: h * 96 + 16] *= SCALE          # fold softmax scale into Wq
    wt = np.ascontiguousarray(w_s.T).astype(NPBF)  # [384, 1152]
    in_p = []
    for c in range(8):
        b, half = divmod(c, 2)
        in_p.append({"ft": np.ascontiguousarray(featT[b][:, half * TPB:(half + 1) * TPB]),
                     "wt": wt})
    t1 = time.time()
    res_p = run_bass_kernel_spmd(_get("p"), in_p, core_ids=list(range(8)))
    t2 = time.time()

    # ---- host gather into cluster order + augmented rows
    qkv_all = [np.concatenate([res_p.results[2 * b]["qkv"],
                               res_p.results[2 * b + 1]["qkv"]], axis=1)
               for b in range(B)]                  # [1152, N] bf16 each
    pos_n = pos / pos.reshape(-1, D).max(0)
    b_of = np.repeat(np.arange(B), H)
    pos_g = np.take_along_axis(pos_n[b_of], mf[:, :, None], axis=1)   # [48,N,2]
    s_g = np.einsum('rnd,rd->rn', pos_g, np.tile(w_pos, (B, 1))).astype(np.float32)

    ones = np.ones((N,), NPBF)
    zeros = np.zeros((N,), NPBF)
    has_bias = bool(np.any(b_qkv))
    qk_host = np.empty((8, R * 40, N), NPBF)
    vt_host = np.empty((8, R * 128, 64 * 65), NPBF)
    for r in range(BH):
        b, h = divmod(r, H)
        core, rr = divmod(r, R)
        blk = qkv_all[b]
        idx = mf[r]
        qg = blk[h * 96: h * 96 + 16][:, idx]
        kg = blk[h * 96 + 16: h * 96 + 32][:, idx]
        vg = blk[h * 96 + 32: h * 96 + 96][:, idx]          # [64, N] bf16
        row2 = (-s_g[r]).astype(NPBF)
        row5 = (s_g[r] + b_pos[h]).astype(NPBF)
        rowqA, rowkB = zeros, zeros
        if has_bias:
            bq = b_qkv[h * 96: h * 96 + 16]
            bk = b_qkv[h * 96 + 16: h * 96 + 32]
            # qg already carries SCALE, so bk @ qg == scale*(bk . q_raw)
            rowqA = (bk @ qg.astype(np.float32)).astype(NPBF)
            row5 = (s_g[r] + b_pos[h] + SCALE * (bq @ kg.astype(np.float32))
                    + SCALE * float(bq @ bk)).astype(NPBF)
        qa = qk_host[core, rr * 40: rr * 40 + 20]
        qa[0:16] = qg; qa[16] = rowqA; qa[17] = ones; qa[18] = row2; qa[19] = ones
        ka = qk_host[core, rr * 40 + 20: rr * 40 + 40]
        ka[0:16] = kg; ka[16] = ones; ka[17] = row5; ka[18] = ones; ka[19] = rowkB
        vt = np.empty((N, 65), NPBF)
        vt[:, 0:64] = vg.T
        if has_bias:
            bv = np.concatenate([b_qkv[h * 96 + 32: h * 96 + 96]])
            vt[:, 0:64] = (vt[:, 0:64].astype(np.float32) + bv).astype(NPBF)
        vt[:, 64] = 1.0
        vt_host[core, rr * 128:(rr + 1) * 128] = (
            vt.reshape(64, 128, 65).transpose(1, 0, 2).reshape(128, 64 * 65))
    in_a = [{"qk": qk_host[c], "vt": vt_host[c]} for c in range(8)]
    t3 = time.time()
    res_a = run_bass_kernel_spmd(_get("a"), in_a, core_ids=list(range(8)))
    t4 = time.time()

    # ---- host: normalize, scatter to token order, build f2T
    f2T = np.empty((B, 2 * C, N), NPBF)
    for r in range(BH):
        b, h = divmod(r, H)
        core, rr = divmod(r, R)
        o = res_a.results[core]["og"][rr * 65:(rr + 1) * 65].astype(np.float32)
        on = o[0:64] / o[64:65]
        f2T[b][h * 64:(h + 1) * 64][:, mf[r]] = on.astype(NPBF)
    wp2 = np.ascontiguousarray(w_proj.T).astype(NPBF)       # [768, 384]
    b_eff = b_proj + w_proj @ np.concatenate(
        [b_qkv[h * 96 + 32: h * 96 + 96] for h in range(H)])
    in_b = []
    for c in range(8):
        b, half = divmod(c, 2)
        in_b.append({
            "f2T": np.ascontiguousarray(f2T[b][:, half * TPB:(half + 1) * TPB]),
            "wp2": wp2,
            "bias": b_eff.reshape(384, 1).astype(np.float32),
        })
    t5 = time.time()
    res_b = run_bass_kernel_spmd(_get("b"), in_b, core_ids=list(range(8)))
    t6 = time.time()

    out = np.empty((B, N, C), np.float32)
    for c in range(8):
        b, half = divmod(c, 2)
        out[b, half * TPB:(half + 1) * TPB, :] = \
            res_b.results[c]["outT"].astype(np.float32).T
    if os.environ.get("KTIME"):
        print(f"[kernel] prep1={t1-t0:.2f}s runP={t2-t1:.2f}s prep2={t3-t2:.2f}s "
              f"runA={t4-t3:.2f}s prep3={t5-t4:.2f}s runB={t6-t5:.2f}s")
    return out


# revision 53
# speedup vs baseline: 4.6323x; 1.0002x over previous
"""ClusterAttention Trainium2 kernel — 3-phase design.

Phase P (proj): token-order qkv projection, shared across heads.
  Each core handles (b, token-half): qkv[1152, 4096] = W^T-chunks.T @ featT,
  o-major, bf16 in/out, fp32 psum. Host pre-scales Wq rows by softmax scale.
Host gather: per (b,h) row, gather q/k/v columns into cluster order, build
  augmented q/k (20 rows: 16 qk dims + bias/pos-bias fold rows) and t-major
  v with a ones column (softmax denominator via matmul).
Phase A (attention): per core 6 rows; per cluster S'=k_aug.T@q_aug ->
  exp on ACT (psum->sbuf bf16) -> transposed AV: O[c,i] = sum_j v_t[j,c]E[j,i]
  with 256-wide moving dim; row 64 of O = denominator. Out o-major, bf16,
  unnormalized (host divides by denominator).
Host scatter: normalize, scatter to token order, build feat2T per (b, half).
Phase B (proj): outT[384, 4096] = w_proj chunks.T @ feat2T + bias.

All matmuls bf16 (1 cycle/col vs 4 for fp32); psum->sbuf copies spread over
DVE and ACT; big out DMAs ride the idle Pool SWDGE queue; all DMAs keep
>=512B contiguous runs (below that the model halves DMA bandwidth).
"""
import os
import numpy as np
import ml_dtypes

import concourse.bacc as bacc
import concourse.tile as tile
from concourse import mybir
from concourse.bass_utils import run_bass_kernel_spmd

B, N, C, H, D, K, M = 4, 8192, 384, 12, 2, 32, 256
CH = C // H // 2            # 16
BH = B * H                  # 48
R = BH // 8                 # 6 rows (heads) per core in phase A
SCALE = float((C // H) ** -0.5)
TPB = N * B // 8            # 4096 tokens per core in phases P and B

F32 = mybir.dt.float32
BF16 = mybir.dt.bfloat16
NPBF = ml_dtypes.bfloat16
EXP = mybir.ActivationFunctionType.Exp
# copy-engine set: d=DVE, s=ACT(scalar), g=Pool(gpsimd, breaks device lowering);
# round-robined in order
KCOPY = os.environ.get("KCOPY", "sd")
BOUT = os.environ.get("BOUT", "bf16")  # phase-B output dtype


def _copy_psum(nc, dst, src, i, eng=None):
    eng = eng or KCOPY
    c = eng[i % len(eng)]
    if c == "d":
        nc.vector.tensor_copy(dst, src)
    elif c == "s":
        nc.scalar.copy(dst, src)
    else:
        nc.gpsimd.tensor_copy(dst, src)


def build_phase_p():
    nc = bacc.Bacc(None, target_bir_lowering=False)
    ft = nc.dram_tensor("ft", [3 * 128, TPB], BF16, kind="ExternalInput")
    wt = nc.dram_tensor("wt", [3 * 128, 9 * 128], BF16, kind="ExternalInput")
    qkv = nc.dram_tensor("qkv", [9 * 128, TPB], BF16, kind="ExternalOutput")
    with tile.TileContext(nc) as tc:
        with (
            tc.tile_pool(name="sb", bufs=1) as pool,
            tc.tile_pool(name="sb_o", bufs=1) as p_o,
            tc.tile_pool(name="ps", bufs=4, space="PSUM") as ps,
        ):
            ft_sb = pool.tile([128, 3 * TPB], BF16, tag="ft")
            w_sb = pool.tile([128, 3 * 1152], BF16, tag="w")
            warm = pool.tile([20, 256], BF16, tag="warm")
            nc.vector.memset(warm[:, :], 0.0)
            wps = ps.tile([128, 512], F32, tag="p")
            for i in range(10):
                nc.tensor.matmul(wps[:, 0:256], warm[:, 0:128], warm[:, 0:256],
                                 start=True, stop=True)
            # fused DMAs (3 cc chunks in one 3D access pattern each)
            ftr = ft.rearrange("(c p) t -> p c t", p=128)
            ftv = ft_sb.rearrange("p (c t) -> p c t", t=TPB)
            wtr = wt.rearrange("(c p) j -> p c j", p=128)
            wtv = w_sb.rearrange("p (c j) -> p c j", j=1152)
            # oc=0 weight slice first so the first matmul isn't gated on all of w
            nc.sync.dma_start(wtv[:, :, 0:256], wtr[:, :, 0:256])
            nc.sync.dma_start(ftv[:, :, 0:512], ftr[:, :, 0:512])
            nc.sync.dma_start(wtv[:, :, 256:], wtr[:, :, 256:])
            for q0 in range(512, TPB, 512):
                nc.sync.dma_start(ftv[:, :, q0:q0 + 512], ftr[:, :, q0:q0 + 512])
            o_all = pool.tile([128, 9 * TPB], BF16, tag="o_all")
            o_v = o_all.rearrange("p (c t) -> p c t", t=TPB)
            qkv_v = qkv.rearrange("(c p) t -> p c t", p=128)
            # token tiles taper at the end so the final drain DMA is tiny;
            # fused 3D out DMAs (all 9 oc at once) keep the HWDGE count low
            widths = [512] * 7 + [256, 256]
            drains = [(0, 1024), (1024, 1024), (2048, 1024), (3072, 512),
                      (3584, 256), (3840, 256)]
            t0 = 0
            di = 0
            for tt, w in enumerate(widths):
                for oc in range(9):
                    p = ps.tile([128, 512], F32, tag="p")
                    for cc in range(3):
                        nc.tensor.matmul(
                            p[:, 0:w],
                            w_sb[:, cc * 1152 + oc * 128: cc * 1152 + (oc + 1) * 128],
                            ft_sb[:, cc * TPB + t0: cc * TPB + t0 + w],
                            start=(cc == 0), stop=(cc == 2))
                    _copy_psum(nc, o_v[:, oc, t0:t0 + w], p[:, 0:w], oc)
                t0 += w
                while di < len(drains) and drains[di][0] + drains[di][1] <= t0:
                    d0, dw = drains[di]
                    di += 1
                    nc.sync.dma_start(qkv_v[:, :, d0:d0 + dw],
                                      o_v[:, :, d0:d0 + dw])
    nc.compile()
    return nc


def build_phase_a():
    nc = bacc.Bacc(None, target_bir_lowering=False)
    qk = nc.dram_tensor("qk", [R * 40, N], BF16, kind="ExternalInput")
    vt = nc.dram_tensor("vt", [R * 128, 64 * 65], BF16, kind="ExternalInput")
    og = nc.dram_tensor("og", [R * 65, N], BF16, kind="ExternalOutput")
    with tile.TileContext(nc) as tc:
        with (
            tc.tile_pool(name="row", bufs=2) as p_row,
            tc.tile_pool(name="e", bufs=int(os.environ.get("KEB", "5"))) as p_e,
            tc.tile_pool(name="ps_s", bufs=(2 if os.environ.get("KGRP", "3") == "3" else 3), space="PSUM") as ps_s,
            tc.tile_pool(name="ps_o", bufs=2, space="PSUM") as ps_o,
        ):
            NP_ = K // 2  # cluster pairs per row
            rows = {}

            # warm the ACT exp table at t=0 so the 1.3us load hides under DMA,
            # and run dummy matmuls so the PE p-state ramps while DMAs fly
            scratch = p_e.tile([1, 8], F32, tag="warm")
            nc.vector.memset(scratch[:, :], 0.0)
            nc.scalar.activation(scratch[:, 4:8], scratch[:, 0:4], EXP)
            wsb = p_e.tile([20, 256], BF16, tag="wsb")
            nc.vector.memset(wsb[:, :], 0.0)
            wps = ps_o.tile([128, 512], F32, tag="po")
            for i in range(14):
                nc.tensor.matmul(wps[:, 0:256], wsb[:, 0:128], wsb[:, 0:256],
                                 start=True, stop=True)

            def load_row(r):
                q_sb = p_row.tile([20, N], BF16, tag="q")
                k_sb = p_row.tile([20, N], BF16, tag="k")
                v_sb = p_row.tile([128, 64 * 65], BF16, tag="v")
                o_sb = p_row.tile([65, N], BF16, tag="o")
                # chunked so the first clusters' operands land early
                for lo, hi in ((0, 256), (256, 2048), (2048, N)):
                    nc.sync.dma_start(q_sb[:, lo:hi], qk[r * 40: r * 40 + 20, lo:hi])
                    nc.sync.dma_start(k_sb[:, lo:hi],
                                      qk[r * 40 + 20: r * 40 + 40, lo:hi])
                nc.sync.dma_start(v_sb[:, 0: 8 * 65],
                                  vt[r * 128:(r + 1) * 128, 0: 8 * 65])
                nc.sync.dma_start(v_sb[:, 8 * 65:],
                                  vt[r * 128:(r + 1) * 128, 8 * 65:])
                rows[r] = (q_sb, k_sb, v_sb.rearrange("p (c w) -> p c w", w=65), o_sb)

            # Flat software pipeline over all clusters: QK+exp runs DEPTH_C
            # clusters ahead of AV+copy so ACT (the bottleneck) never starves.
            # S/E tiles batch 3 clusters ([128, 1536], 3 psum banks) to cut the
            # per-activation init overhead; AV works in 2-cluster units.
            # Input DMAs prefetch one row ahead on SP; out DMAs stream via the
            # idle Pool SWDGE queue, except each row's last two chunks which
            # ride SP/inline so the program tail is one small hop.
            DEPTH_C = int(os.environ.get("KDC", "8"))
            GROUPS = ([3] * 10 + [2]) if os.environ.get("KGRP", "3") == "3" else [2] * 16
            sitems = []
            for r in range(R):
                c0 = 0
                for g in GROUPS:
                    sitems.append((r, c0, g))
                    c0 += g
            e_of = {}
            av_list = [(r, pp) for r in range(R) for pp in range(NP_)]
            av_ptr = 0
            issued = 0
            done = 0

            def do_av(eng="d"):
                nonlocal av_ptr, done
                qr, qp = av_list[av_ptr]
                av_ptr += 1
                done += 2
                _, _, v_view, o_sb = rows[qr]
                po = ps_o.tile([128, 512], F32, tag="po")
                for u in range(2):
                    et, off = e_of.pop((qr, qp * 2 + u))
                    for jc in range(2):
                        nc.tensor.matmul(
                            po[0:65, u * 256:(u + 1) * 256],
                            v_view[:, (qp * 2 + u) * 2 + jc, 0:65],
                            et[:, off + jc * 256: off + (jc + 1) * 256],
                            start=(jc == 0), stop=(jc == 1))
                _copy_psum(nc, o_sb[:, qp * 512:(qp + 1) * 512],
                           po[0:65, :], 0, eng=eng)
                # stream the row's output: big chunks via Pool SWDGE, the last
                # two pairs per-chunk on SP right behind their copies
                if qp == 7:
                    nc.gpsimd.dma_start(og[qr * 65:(qr + 1) * 65, 0:4096],
                                        o_sb[:, 0:4096])
                elif qp == 13:
                    nc.gpsimd.dma_start(og[qr * 65:(qr + 1) * 65, 4096:7168],
                                        o_sb[:, 4096:7168])
                elif qp >= 14:
                    nc.sync.dma_start(
                        og[qr * 65:(qr + 1) * 65, qp * 512:(qp + 1) * 512],
                        o_sb[:, qp * 512:(qp + 1) * 512])

            load_row(0)
            for r, c0, g in sitems:
                if c0 == 0 and r + 1 < R:
                    load_row(r + 1)
                # drain AV work first so PE has queued work while the next
                # S-group's psum recycles through the pending exp; taper the
                # lag over the last groups so the end-flush backlog is small
                lag = DEPTH_C + g
                left = len(sitems) - sitems.index((r, c0, g))
                if left <= 3:
                    lag = min(lag, 2 * left)
                while av_ptr < len(av_list) and issued - done >= lag:
                    do_av()
                q_sb, k_sb, v_view, o_sb = rows[r]
                ps = ps_s.tile([128, 512 * g], F32, tag="s",
                               padded_shape=[128, 512 * max(GROUPS)])
                for i in range(g):
                    col = (c0 + i) * 256
                    nc.tensor.matmul(ps[:, i * 512: i * 512 + 256],
                                     k_sb[:, col:col + 128],
                                     q_sb[:, col:col + 256],
                                     start=True, stop=True)
                    nc.tensor.matmul(ps[:, i * 512 + 256: i * 512 + 512],
                                     k_sb[:, col + 128:col + 256],
                                     q_sb[:, col:col + 256],
                                     start=True, stop=True)
                e = p_e.tile([128, 512 * g], BF16, tag="e",
                             padded_shape=[128, 512 * max(GROUPS)])
                nc.scalar.activation(e[:, :], ps[:, :], EXP)
                for i in range(g):
                    e_of[(r, c0 + i)] = (e, i * 512)
                issued += g
            flush_i = 0
            while av_ptr < len(av_list):
                do_av(eng="ds"[flush_i % 2])
                flush_i += 1
    nc.compile()
    return nc


def build_phase_b():
    dt_out = F32 if BOUT == "f32" else BF16
    nc = bacc.Bacc(None, target_bir_lowering=False)
    f2T = nc.dram_tensor("f2T", [6 * 128, TPB], BF16, kind="ExternalInput")
    wp2 = nc.dram_tensor("wp2", [6 * 128, 384], BF16, kind="ExternalInput")
    bias = nc.dram_tensor("bias", [3 * 128, 1], F32, kind="ExternalInput")
    outT = nc.dram_tensor("outT", [3 * 128, TPB], dt_out, kind="ExternalOutput")
    with tile.TileContext(nc) as tc:
        with (
            tc.tile_pool(name="sb", bufs=1) as pool,
            tc.tile_pool(name="sb_o", bufs=1) as p_o,
            tc.tile_pool(name="ps", bufs=4, space="PSUM") as ps,
        ):
            fsb = pool.tile([128, 6 * TPB], BF16, tag="fsb")
            wsb = pool.tile([128, 6 * 384], BF16, tag="wsb")
            bsb = pool.tile([128, 3], F32, tag="bsb")
            warm = pool.tile([20, 256], BF16, tag="warm")
            nc.vector.memset(warm[:, :], 0.0)
            wps = ps.tile([128, 512], F32, tag="p")
            for i in range(12):
                nc.tensor.matmul(wps[:, 0:256], warm[:, 0:128], warm[:, 0:256],
                                 start=True, stop=True)
            # fused DMAs (6 cc chunks in one 3D access pattern each)
            nc.sync.dma_start(
                bsb.rearrange("p (c j) -> p c j", j=1)[:, :, :],
                bias.rearrange("(c p) j -> p c j", p=128)[:, :, :])
            fr = f2T.rearrange("(c p) t -> p c t", p=128)
            fv = fsb.rearrange("p (c t) -> p c t", t=TPB)
            wr = wp2.rearrange("(c p) j -> p c j", p=128)
            wv = wsb.rearrange("p (c j) -> p c j", j=384)
            nc.sync.dma_start(wv[:, :, 0:256], wr[:, :, 0:256])
            nc.sync.dma_start(fv[:, :, 0:256], fr[:, :, 0:256])
            nc.sync.dma_start(wv[:, :, 256:], wr[:, :, 256:])
            nc.sync.dma_start(fv[:, :, 256:512], fr[:, :, 256:512])
            for q0 in range(512, TPB, 512):
                nc.sync.dma_start(fv[:, :, q0:q0 + 512], fr[:, :, q0:q0 + 512])
            o_all = pool.tile([128, 3 * TPB], dt_out, tag="o_all")
            o_v = o_all.rearrange("p (c t) -> p c t", t=TPB)
            out_v = outT.rearrange("(c p) t -> p c t", p=128)
            widths = [256, 256] + [512] * 6 + [256, 256]
            drains = [(0, 512), (512, 1024), (1536, 1024), (2560, 1024),
                      (3584, 256), (3840, 256)]
            t0 = 0
            di = 0
            for tt, w in enumerate(widths):
                for oc in range(3):
                    p = ps.tile([128, 512], F32, tag="p")
                    for cc in range(6):
                        nc.tensor.matmul(
                            p[:, 0:w],
                            wsb[:, cc * 384 + oc * 128: cc * 384 + (oc + 1) * 128],
                            fsb[:, cc * TPB + t0: cc * TPB + t0 + w],
                            start=(cc == 0), stop=(cc == 5))
                    nc.vector.tensor_scalar(o_v[:, oc, t0:t0 + w], p[:, 0:w],
                                            bsb[:, oc:oc + 1], None,
                                            mybir.AluOpType.add)
                t0 += w
                while di < len(drains) and drains[di][0] + drains[di][1] <= t0:
                    d0, dw = drains[di]
                    di += 1
                    nc.sync.dma_start(out_v[:, :, d0:d0 + dw],
                                      o_v[:, :, d0:d0 + dw])
    nc.compile()
    return nc


_CACHE = {}
PHASES = ("p", "a", "b")
_BUILDERS = {"p": build_phase_p, "a": build_phase_a, "b": build_phase_b}


def _get(name):
    if name not in _CACHE:
        _CACHE[name] = _BUILDERS[name]()
    return _CACHE[name]


def kernel(pos, feat, member_idx, w_qkv, b_qkv, w_pos, b_pos, w_proj, b_proj):
    import time
    pos = np.asarray(pos, np.float32)
    feat = np.asarray(feat, np.float32)
    mf = np.asarray(member_idx).astype(np.int64).reshape(BH, N)
    w_qkv = np.asarray(w_qkv, np.float32); b_qkv = np.asarray(b_qkv, np.float32)
    w_pos = np.asarray(w_pos, np.float32); b_pos = np.asarray(b_pos, np.float32)
    w_proj = np.asarray(w_proj, np.float32); b_proj = np.asarray(b_proj, np.float32)

    t0 = time.time()
    # ---- phase P host prep: featT per (b, half), prescaled w_qkv^T
    featT = np.ascontiguousarray(feat.transpose(0, 2, 1)).astype(NPBF)  # [B,C,N]
    w_s = w_qkv.copy()
    for h in range(H):
        w_s[h * 96: h * 96 + 16] *= SCALE          # fold softmax scale into Wq
    wt = np.ascontiguousarray(w_s.T).astype(NPBF)  # [384, 1152]
    in_p = []
    for c in range(8):
        b, half = divmod(c, 2)
        in_p.append({"ft": np.ascontiguousarray(featT[b][:, half * TPB:(half + 1) * TPB]),
                     "wt": wt})
    t1 = time.time()
    res_p = run_bass_kernel_spmd(_get("p"), in_p, core_ids=list(range(8)))
    t2 = time.time()

    # ---- host gather into cluster order + augmented rows
    qkv_all = [np.concatenate([res_p.results[2 * b]["qkv"],
                               res_p.results[2 * b + 1]["qkv"]], axis=1)
               for b in range(B)]                  # [1152, N] bf16 each
    pos_n = pos / pos.reshape(-1, D).max(0)
    b_of = np.repeat(np.arange(B), H)
    pos_g = np.take_along_axis(pos_n[b_of], mf[:, :, None], axis=1)   # [48,N,2]
    s_g = np.einsum('rnd,rd->rn', pos_g, np.tile(w_pos, (B, 1))).astype(np.float32)

    ones = np.ones((N,), NPBF)
    zeros = np.zeros((N,), NPBF)
    has_bias = bool(np.any(b_qkv))
    qk_host = np.empty((8, R * 40, N), NPBF)
    vt_host = np.empty((8, R * 128, 64 * 65), NPBF)
    for r in range(BH):
        b, h = divmod(r, H)
        core, rr = divmod(r, R)
        blk = qkv_all[b]
        idx = mf[r]
        qg = blk[h * 96: h * 96 + 16][:, idx]
        kg = blk[h * 96 + 16: h * 96 + 32][:, idx]
        vg = blk[h * 96 + 32: h * 96 + 96][:, idx]          # [64, N] bf16
        row2 = (-s_g[r]).astype(NPBF)
        row5 = (s_g[r] + b_pos[h]).astype(NPBF)
        rowqA, rowkB = zeros, zeros
        if has_bias:
            bq = b_qkv[h * 96: h * 96 + 16]
            bk = b_qkv[h * 96 + 16: h * 96 + 32]
            # qg already carries SCALE, so bk @ qg == scale*(bk . q_raw)
            rowqA = (bk @ qg.astype(np.float32)).astype(NPBF)
            row5 = (s_g[r] + b_pos[h] + SCALE * (bq @ kg.astype(np.float32))
                    + SCALE * float(bq @ bk)).astype(NPBF)
        qa = qk_host[core, rr * 40: rr * 40 + 20]
        qa[0:16] = qg; qa[16] = rowqA; qa[17] = ones; qa[18] = row2; qa[19] = ones
        ka = qk_host[core, rr * 40 + 20: rr * 40 + 40]
        ka[0:16] = kg; ka[16] = ones; ka[17] = row5; ka[18] = ones; ka[19] = rowkB
        vt = np.empty((N, 65), NPBF)
        vt[:, 0:64] = vg.T
        if has_bias:
            bv = np.concatenate([b_qkv[h * 96 + 32: h * 96 + 96]])
            vt[:, 0:64] = (vt[:, 0:64].astype(np.float32) + bv).astype(NPBF)
        vt[:, 64] = 1.0
        vt_host[core, rr * 128:(rr + 1) * 128] = (
            vt.reshape(64, 128, 65).transpose(1, 0, 2).reshape(128, 64 * 65))
    in_a = [{"qk": qk_host[c], "vt": vt_host[c]} for c in range(8)]
    t3 = time.time()
    res_a = run_bass_kernel_spmd(_get("a"), in_a, core_ids=list(range(8)))
    t4 = time.time()

    # ---- host: normalize, scatter to token order, build f2T
    f2T = np.empty((B, 2 * C, N), NPBF)
    for r in range(BH):
        b, h = divmod(r, H)
        core, rr = divmod(r, R)
        o = res_a.results[core]["og"][rr * 65:(rr + 1) * 65].astype(np.float32)
        on = o[0:64] / o[64:65]
        f2T[b][h * 64:(h + 1) * 64][:, mf[r]] = on.astype(NPBF)
    wp2 = np.ascontiguousarray(w_proj.T).astype(NPBF)       # [768, 384]
    b_eff = b_proj + w_proj @ np.concatenate(
        [b_qkv[h * 96 + 32: h * 96 + 96] for h in range(H)])
    in_b = []
    for c in range(8):
        b, half = divmod(c, 2)
        in_b.append({
            "f2T": np.ascontiguousarray(f2T[b][:, half * TPB:(half + 1) * TPB]),
            "wp2": wp2,
            "bias": b_eff.reshape(384, 1).astype(np.float32),
        })
    t5 = time.time()
    res_b = run_bass_kernel_spmd(_get("b"), in_b, core_ids=list(range(8)))
    t6 = time.time()

    out = np.empty((B, N, C), np.float32)
    for c in range(8):
        b, half = divmod(c, 2)
        out[b, half * TPB:(half + 1) * TPB, :] = \
            res_b.results[c]["outT"].astype(np.float32).T
    if os.environ.get("KTIME"):
        print(f"[kernel] prep1={t1-t0:.2f}s runP={t2-t1:.2f}s prep2={t3-t2:.2f}s "
              f"runA={t4-t3:.2f}s prep3={t5-t4:.2f}s runB={t6-t5:.2f}s")
    return out


# revision 56
# speedup vs baseline: 4.6421x; 1.0021x over previous
"""ClusterAttention Trainium2 kernel — 3-phase design.

Phase P (proj): token-order qkv projection, shared across heads.
  Each core handles (b, token-half): qkv[1152, 4096] = W^T-chunks.T @ featT,
  o-major, bf16 in/out, fp32 psum. Host pre-scales Wq rows by softmax scale.
Host gather: per (b,h) row, gather q/k/v columns into cluster order, build
  augmented q/k (20 rows: 16 qk dims + bias/pos-bias fold rows) and t-major
  v with a ones column (softmax denominator via matmul).
Phase A (attention): per core 6 rows; per cluster S'=k_aug.T@q_aug ->
  exp on ACT (psum->sbuf bf16) -> transposed AV: O[c,i] = sum_j v_t[j,c]E[j,i]
  with 256-wide moving dim; row 64 of O = denominator. Out o-major, bf16,
  unnormalized (host divides by denominator).
Host scatter: normalize, scatter to token order, build feat2T per (b, half).
Phase B (proj): outT[384, 4096] = w_proj chunks.T @ feat2T + bias.

All matmuls bf16 (1 cycle/col vs 4 for fp32); psum->sbuf copies spread over
DVE and ACT; big out DMAs ride the idle Pool SWDGE queue; all DMAs keep
>=512B contiguous runs (below that the model halves DMA bandwidth).
"""
import os
import numpy as np
import ml_dtypes

import concourse.bacc as bacc
import concourse.tile as tile
from concourse import mybir
from concourse.bass_utils import run_bass_kernel_spmd

B, N, C, H, D, K, M = 4, 8192, 384, 12, 2, 32, 256
CH = C // H // 2            # 16
BH = B * H                  # 48
R = BH // 8                 # 6 rows (heads) per core in phase A
SCALE = float((C // H) ** -0.5)
TPB = N * B // 8            # 4096 tokens per core in phases P and B

F32 = mybir.dt.float32
BF16 = mybir.dt.bfloat16
NPBF = ml_dtypes.bfloat16
EXP = mybir.ActivationFunctionType.Exp
# copy-engine set: d=DVE, s=ACT(scalar), g=Pool(gpsimd, breaks device lowering);
# round-robined in order
KCOPY = os.environ.get("KCOPY", "sd")
BOUT = os.environ.get("BOUT", "bf16")  # phase-B output dtype


def _copy_psum(nc, dst, src, i, eng=None):
    eng = eng or KCOPY
    c = eng[i % len(eng)]
    if c == "d":
        nc.vector.tensor_copy(dst, src)
    elif c == "s":
        nc.scalar.copy(dst, src)
    else:
        nc.gpsimd.tensor_copy(dst, src)


def build_phase_p():
    nc = bacc.Bacc(None, target_bir_lowering=False)
    ft = nc.dram_tensor("ft", [3 * 128, TPB], BF16, kind="ExternalInput")
    wt = nc.dram_tensor("wt", [3 * 128, 9 * 128], BF16, kind="ExternalInput")
    qkv = nc.dram_tensor("qkv", [9 * 128, TPB], BF16, kind="ExternalOutput")
    with tile.TileContext(nc) as tc:
        with (
            tc.tile_pool(name="sb", bufs=1) as pool,
            tc.tile_pool(name="sb_o", bufs=1) as p_o,
            tc.tile_pool(name="ps", bufs=4, space="PSUM") as ps,
        ):
            ft_sb = pool.tile([128, 3 * TPB], BF16, tag="ft")
            w_sb = pool.tile([128, 3 * 1152], BF16, tag="w")
            warm = pool.tile([20, 256], BF16, tag="warm")
            nc.vector.memset(warm[:, :], 0.0)
            wps = ps.tile([128, 512], F32, tag="p")
            for i in range(10):
                nc.tensor.matmul(wps[:, 0:256], warm[:, 0:128], warm[:, 0:256],
                                 start=True, stop=True)
            # fused DMAs (3 cc chunks in one 3D access pattern each)
            ftr = ft.rearrange("(c p) t -> p c t", p=128)
            ftv = ft_sb.rearrange("p (c t) -> p c t", t=TPB)
            wtr = wt.rearrange("(c p) j -> p c j", p=128)
            wtv = w_sb.rearrange("p (c j) -> p c j", j=1152)
            # oc=0 weight slice first so the first matmul isn't gated on all of w
            nc.sync.dma_start(wtv[:, :, 0:256], wtr[:, :, 0:256])
            nc.sync.dma_start(ftv[:, :, 0:512], ftr[:, :, 0:512])
            nc.sync.dma_start(wtv[:, :, 256:], wtr[:, :, 256:])
            for q0 in range(512, TPB, 512):
                nc.sync.dma_start(ftv[:, :, q0:q0 + 512], ftr[:, :, q0:q0 + 512])
            o_all = pool.tile([128, 9 * TPB], BF16, tag="o_all")
            o_v = o_all.rearrange("p (c t) -> p c t", t=TPB)
            qkv_v = qkv.rearrange("(c p) t -> p c t", p=128)
            # token tiles taper at the end so the final drain DMA is tiny;
            # fused 3D out DMAs (all 9 oc at once) keep the HWDGE count low
            widths = [512] * 7 + [256, 256]
            drains = [(0, 1024), (1024, 1024), (2048, 1024), (3072, 512),
                      (3584, 256), (3840, 256)]
            t0 = 0
            di = 0
            for tt, w in enumerate(widths):
                for oc in range(9):
                    p = ps.tile([128, 512], F32, tag="p")
                    for cc in range(3):
                        nc.tensor.matmul(
                            p[:, 0:w],
                            w_sb[:, cc * 1152 + oc * 128: cc * 1152 + (oc + 1) * 128],
                            ft_sb[:, cc * TPB + t0: cc * TPB + t0 + w],
                            start=(cc == 0), stop=(cc == 2))
                    _copy_psum(nc, o_v[:, oc, t0:t0 + w], p[:, 0:w], oc)
                t0 += w
                while di < len(drains) and drains[di][0] + drains[di][1] <= t0:
                    d0, dw = drains[di]
                    di += 1
                    nc.sync.dma_start(qkv_v[:, :, d0:d0 + dw],
                                      o_v[:, :, d0:d0 + dw])
    nc.compile()
    return nc


def build_phase_a():
    nc = bacc.Bacc(None, target_bir_lowering=False)
    qk = nc.dram_tensor("qk", [R * 40, N], BF16, kind="ExternalInput")
    vt = nc.dram_tensor("vt", [R * 128, 64 * 65], BF16, kind="ExternalInput")
    og = nc.dram_tensor("og", [R * 65, N], BF16, kind="ExternalOutput")
    with tile.TileContext(nc) as tc:
        with (
            tc.tile_pool(name="row", bufs=2) as p_row,
            tc.tile_pool(name="e", bufs=int(os.environ.get("KEB", "5"))) as p_e,
            tc.tile_pool(name="ps_s", bufs=(3 if os.environ.get("KGRP", "3f") == "2" else 2),
                         space="PSUM") as ps_s,
            tc.tile_pool(name="ps_o", bufs=2, space="PSUM") as ps_o,
        ):
            NP_ = K // 2  # cluster pairs per row
            rows = {}

            # warm the ACT exp table at t=0 so the 1.3us load hides under DMA,
            # and run dummy matmuls so the PE p-state ramps while DMAs fly
            scratch = p_e.tile([1, 8], F32, tag="warm")
            nc.vector.memset(scratch[:, :], 0.0)
            nc.scalar.activation(scratch[:, 4:8], scratch[:, 0:4], EXP)
            wsb = p_e.tile([20, 256], BF16, tag="wsb")
            nc.vector.memset(wsb[:, :], 0.0)
            wps = ps_o.tile([128, 512], F32, tag="po")
            for i in range(14):
                nc.tensor.matmul(wps[:, 0:256], wsb[:, 0:128], wsb[:, 0:256],
                                 start=True, stop=True)

            def load_row(r):
                q_sb = p_row.tile([20, N], BF16, tag="q")
                k_sb = p_row.tile([20, N], BF16, tag="k")
                v_sb = p_row.tile([128, 64 * 65], BF16, tag="v")
                o_sb = p_row.tile([65, N], BF16, tag="o")
                # chunked so the first clusters' operands land early
                for lo, hi in ((0, 256), (256, 2048), (2048, N)):
                    nc.sync.dma_start(q_sb[:, lo:hi], qk[r * 40: r * 40 + 20, lo:hi])
                    nc.sync.dma_start(k_sb[:, lo:hi],
                                      qk[r * 40 + 20: r * 40 + 40, lo:hi])
                nc.sync.dma_start(v_sb[:, 0: 8 * 65],
                                  vt[r * 128:(r + 1) * 128, 0: 8 * 65])
                nc.sync.dma_start(v_sb[:, 8 * 65:],
                                  vt[r * 128:(r + 1) * 128, 8 * 65:])
                rows[r] = (q_sb, k_sb, v_sb.rearrange("p (c w) -> p c w", w=65), o_sb)

            # Flat software pipeline over all clusters: QK+exp runs DEPTH_C
            # clusters ahead of AV+copy so ACT (the bottleneck) never starves.
            # S/E tiles batch 3 clusters ([128, 1536], 3 psum banks) to cut the
            # per-activation init overhead; AV works in 2-cluster units.
            # Input DMAs prefetch one row ahead on SP; out DMAs stream via the
            # idle Pool SWDGE queue, except each row's last two chunks which
            # ride SP/inline so the program tail is one small hop.
            DEPTH_C = int(os.environ.get("KDC", "9"))
            _g = os.environ.get("KGRP", "3f")
            GROUPS = {"3": [3] * 10 + [2], "3f": [2] + [3] * 10, "2": [2] * 16}[_g]
            sitems = []
            for r in range(R):
                c0 = 0
                for g in GROUPS:
                    sitems.append((r, c0, g))
                    c0 += g
            e_of = {}
            av_list = [(r, pp) for r in range(R) for pp in range(NP_)]
            av_ptr = 0
            issued = 0
            done = 0

            def do_av(eng="d"):
                nonlocal av_ptr, done
                qr, qp = av_list[av_ptr]
                av_ptr += 1
                done += 2
                _, _, v_view, o_sb = rows[qr]
                po = ps_o.tile([128, 512], F32, tag="po")
                for u in range(2):
                    et, off = e_of.pop((qr, qp * 2 + u))
                    for jc in range(2):
                        nc.tensor.matmul(
                            po[0:65, u * 256:(u + 1) * 256],
                            v_view[:, (qp * 2 + u) * 2 + jc, 0:65],
                            et[:, off + jc * 256: off + (jc + 1) * 256],
                            start=(jc == 0), stop=(jc == 1))
                _copy_psum(nc, o_sb[:, qp * 512:(qp + 1) * 512],
                           po[0:65, :], 0, eng=eng)
                # stream the row's output: big chunks via Pool SWDGE, the last
                # two pairs per-chunk on SP right behind their copies
                if qp == 7:
                    nc.gpsimd.dma_start(og[qr * 65:(qr + 1) * 65, 0:4096],
                                        o_sb[:, 0:4096])
                elif qp == 13:
                    nc.gpsimd.dma_start(og[qr * 65:(qr + 1) * 65, 4096:7168],
                                        o_sb[:, 4096:7168])
                elif qp >= 14:
                    nc.sync.dma_start(
                        og[qr * 65:(qr + 1) * 65, qp * 512:(qp + 1) * 512],
                        o_sb[:, qp * 512:(qp + 1) * 512])

            load_row(0)
            for r, c0, g in sitems:
                if c0 == 0 and r + 1 < R:
                    load_row(r + 1)
                # drain AV work first so PE has queued work while the next
                # S-group's psum recycles through the pending exp; taper the
                # lag over the last groups so the end-flush backlog is small
                lag = DEPTH_C + g
                left = len(sitems) - sitems.index((r, c0, g))
                if left <= 3:
                    lag = min(lag, 2 * left)
                while av_ptr < len(av_list) and issued - done >= lag:
                    do_av()
                q_sb, k_sb, v_view, o_sb = rows[r]
                ps = ps_s.tile([128, 512 * g], F32, tag="s",
                               padded_shape=[128, 512 * max(GROUPS)])
                for i in range(g):
                    col = (c0 + i) * 256
                    nc.tensor.matmul(ps[:, i * 512: i * 512 + 256],
                                     k_sb[:, col:col + 128],
                                     q_sb[:, col:col + 256],
                                     start=True, stop=True)
                    nc.tensor.matmul(ps[:, i * 512 + 256: i * 512 + 512],
                                     k_sb[:, col + 128:col + 256],
                                     q_sb[:, col:col + 256],
                                     start=True, stop=True)
                e = p_e.tile([128, 512 * g], BF16, tag="e",
                             padded_shape=[128, 512 * max(GROUPS)])
                nc.scalar.activation(e[:, :], ps[:, :], EXP)
                for i in range(g):
                    e_of[(r, c0 + i)] = (e, i * 512)
                issued += g
            flush_i = 0
            while av_ptr < len(av_list):
                do_av(eng="ds"[flush_i % 2])
                flush_i += 1
    nc.compile()
    return nc


def build_phase_b():
    dt_out = F32 if BOUT == "f32" else BF16
    nc = bacc.Bacc(None, target_bir_lowering=False)
    f2T = nc.dram_tensor("f2T", [6 * 128, TPB], BF16, kind="ExternalInput")
    wp2 = nc.dram_tensor("wp2", [6 * 128, 384], BF16, kind="ExternalInput")
    bias = nc.dram_tensor("bias", [3 * 128, 1], F32, kind="ExternalInput")
    outT = nc.dram_tensor("outT", [3 * 128, TPB], dt_out, kind="ExternalOutput")
    with tile.TileContext(nc) as tc:
        with (
            tc.tile_pool(name="sb", bufs=1) as pool,
            tc.tile_pool(name="sb_o", bufs=1) as p_o,
            tc.tile_pool(name="ps", bufs=4, space="PSUM") as ps,
        ):
            fsb = pool.tile([128, 6 * TPB], BF16, tag="fsb")
            wsb = pool.tile([128, 6 * 384], BF16, tag="wsb")
            bsb = pool.tile([128, 3], F32, tag="bsb")
            warm = pool.tile([20, 256], BF16, tag="warm")
            nc.vector.memset(warm[:, :], 0.0)
            wps = ps.tile([128, 512], F32, tag="p")
            for i in range(12):
                nc.tensor.matmul(wps[:, 0:256], warm[:, 0:128], warm[:, 0:256],
                                 start=True, stop=True)
            # fused DMAs (6 cc chunks in one 3D access pattern each)
            nc.sync.dma_start(
                bsb.rearrange("p (c j) -> p c j", j=1)[:, :, :],
                bias.rearrange("(c p) j -> p c j", p=128)[:, :, :])
            fr = f2T.rearrange("(c p) t -> p c t", p=128)
            fv = fsb.rearrange("p (c t) -> p c t", t=TPB)
            wr = wp2.rearrange("(c p) j -> p c j", p=128)
            wv = wsb.rearrange("p (c j) -> p c j", j=384)
            nc.sync.dma_start(wv[:, :, 0:256], wr[:, :, 0:256])
            nc.sync.dma_start(fv[:, :, 0:256], fr[:, :, 0:256])
            nc.sync.dma_start(wv[:, :, 256:], wr[:, :, 256:])
            nc.sync.dma_start(fv[:, :, 256:512], fr[:, :, 256:512])
            for q0 in range(512, TPB, 512):
                nc.sync.dma_start(fv[:, :, q0:q0 + 512], fr[:, :, q0:q0 + 512])
            o_all = pool.tile([128, 3 * TPB], dt_out, tag="o_all")
            o_v = o_all.rearrange("p (c t) -> p c t", t=TPB)
            out_v = outT.rearrange("(c p) t -> p c t", p=128)
            widths = [256, 256] + [512] * 6 + [256, 256]
            drains = [(0, 512), (512, 1024), (1536, 1024), (2560, 1024),
                      (3584, 256), (3840, 256)]
            t0 = 0
            di = 0
            for tt, w in enumerate(widths):
                for oc in range(3):
                    p = ps.tile([128, 512], F32, tag="p")
                    for cc in range(6):
                        nc.tensor.matmul(
                            p[:, 0:w],
                            wsb[:, cc * 384 + oc * 128: cc * 384 + (oc + 1) * 128],
                            fsb[:, cc * TPB + t0: cc * TPB + t0 + w],
                            start=(cc == 0), stop=(cc == 5))
                    nc.vector.tensor_scalar(o_v[:, oc, t0:t0 + w], p[:, 0:w],
                                            bsb[:, oc:oc + 1], None,
                                            mybir.AluOpType.add)
                t0 += w
                while di < len(drains) and drains[di][0] + drains[di][1] <= t0:
                    d0, dw = drains[di]
                    di += 1
                    nc.sync.dma_start(out_v[:, :, d0:d0 + dw],
                                      o_v[:, :, d0:d0 + dw])
    nc.compile()
    return nc


_CACHE = {}
PHASES = ("p", "a", "b")
_BUILDERS = {"p": build_phase_p, "a": build_phase_a, "b": build_phase_b}


def _get(name):
    if name not in _CACHE:
        _CACHE[name] = _BUILDERS[name]()
    return _CACHE[name]


def kernel(pos, feat, member_idx, w_qkv, b_qkv, w_pos, b_pos, w_proj, b_proj):
    import time
    pos = np.asarray(pos, np.float32)
    feat = np.asarray(feat, np.float32)
    mf = np.asarray(member_idx).astype(np.int64).reshape(BH, N)
    w_qkv = np.asarray(w_qkv, np.float32); b_qkv = np.asarray(b_qkv, np.float32)
    w_pos = np.asarray(w_pos, np.float32); b_pos = np.asarray(b_pos, np.float32)
    w_proj = np.asarray(w_proj, np.float32); b_proj = np.asarray(b_proj, np.float32)

    t0 = time.time()
    # ---- phase P host prep: featT per (b, half), prescaled w_qkv^T
    featT = np.ascontiguousarray(feat.transpose(0, 2, 1)).astype(NPBF)  # [B,C,N]
    w_s = w_qkv.copy()
    for h in range(H):
        w_s[h * 96: h * 96 + 16] *= SCALE          # fold softmax scale into Wq
    wt = np.ascontiguousarray(w_s.T).astype(NPBF)  # [384, 1152]
    in_p = []
    for c in range(8):
        b, half = divmod(c, 2)
        in_p.append({"ft": np.ascontiguousarray(featT[b][:, half * TPB:(half + 1) * TPB]),
                     "wt": wt})
    t1 = time.time()
    res_p = run_bass_kernel_spmd(_get("p"), in_p, core_ids=list(range(8)))
    t2 = time.time()

    # ---- host gather into cluster order + augmented rows
    qkv_all = [np.concatenate([res_p.results[2 * b]["qkv"],
                               res_p.results[2 * b + 1]["qkv"]], axis=1)
               for b in range(B)]                  # [1152, N] bf16 each
    pos_n = pos / pos.reshape(-1, D).max(0)
    b_of = np.repeat(np.arange(B), H)
    pos_g = np.take_along_axis(pos_n[b_of], mf[:, :, None], axis=1)   # [48,N,2]
    s_g = np.einsum('rnd,rd->rn', pos_g, np.tile(w_pos, (B, 1))).astype(np.float32)

    ones = np.ones((N,), NPBF)
    zeros = np.zeros((N,), NPBF)
    has_bias = bool(np.any(b_qkv))
    qk_host = np.empty((8, R * 40, N), NPBF)
    vt_host = np.empty((8, R * 128, 64 * 65), NPBF)
    for r in range(BH):
        b, h = divmod(r, H)
        core, rr = divmod(r, R)
        blk = qkv_all[b]
        idx = mf[r]
        qg = blk[h * 96: h * 96 + 16][:, idx]
        kg = blk[h * 96 + 16: h * 96 + 32][:, idx]
        vg = blk[h * 96 + 32: h * 96 + 96][:, idx]          # [64, N] bf16
        row2 = (-s_g[r]).astype(NPBF)
        row5 = (s_g[r] + b_pos[h]).astype(NPBF)
        rowqA, rowkB = zeros, zeros
        if has_bias:
            bq = b_qkv[h * 96: h * 96 + 16]
            bk = b_qkv[h * 96 + 16: h * 96 + 32]
            # qg already carries SCALE, so bk @ qg == scale*(bk . q_raw)
            rowqA = (bk @ qg.astype(np.float32)).astype(NPBF)
            row5 = (s_g[r] + b_pos[h] + SCALE * (bq @ kg.astype(np.float32))
                    + SCALE * float(bq @ bk)).astype(NPBF)
        qa = qk_host[core, rr * 40: rr * 40 + 20]
        qa[0:16] = qg; qa[16] = rowqA; qa[17] = ones; qa[18] = row2; qa[19] = ones
        ka = qk_host[core, rr * 40 + 20: rr * 40 + 40]
        ka[0:16] = kg; ka[16] = ones; ka[17] = row5; ka[18] = ones; ka[19] = rowkB
        vt = np.empty((N, 65), NPBF)
        vt[:, 0:64] = vg.T
        if has_bias:
            bv = np.concatenate([b_qkv[h * 96 + 32: h * 96 + 96]])
            vt[:, 0:64] = (vt[:, 0:64].astype(np.float32) + bv).astype(NPBF)
        vt[:, 64] = 1.0
        vt_host[core, rr * 128:(rr + 1) * 128] = (
            vt.reshape(64, 128, 65).transpose(1, 0, 2).reshape(128, 64 * 65))
    in_a = [{"qk": qk_host[c], "vt": vt_host[c]} for c in range(8)]
    t3 = time.time()
    res_a = run_bass_kernel_spmd(_get("a"), in_a, core_ids=list(range(8)))
    t4 = time.time()

    # ---- host: normalize, scatter to token order, build f2T
    f2T = np.empty((B, 2 * C, N), NPBF)
    for r in range(BH):
        b, h = divmod(r, H)
        core, rr = divmod(r, R)
        o = res_a.results[core]["og"][rr * 65:(rr + 1) * 65].astype(np.float32)
        on = o[0:64] / o[64:65]
        f2T[b][h * 64:(h + 1) * 64][:, mf[r]] = on.astype(NPBF)
    wp2 = np.ascontiguousarray(w_proj.T).astype(NPBF)       # [768, 384]
    b_eff = b_proj + w_proj @ np.concatenate(
        [b_qkv[h * 96 + 32: h * 96 + 96] for h in range(H)])
    in_b = []
    for c in range(8):
        b, half = divmod(c, 2)
        in_b.append({
            "f2T": np.ascontiguousarray(f2T[b][:, half * TPB:(half + 1) * TPB]),
            "wp2": wp2,
            "bias": b_eff.reshape(384, 1).astype(np.float32),
        })
    t5 = time.time()
    res_b = run_bass_kernel_spmd(_get("b"), in_b, core_ids=list(range(8)))
    t6 = time.time()

    out = np.empty((B, N, C), np.float32)
    for c in range(8):
        b, half = divmod(c, 2)
        out[b, half * TPB:(half + 1) * TPB, :] = \
            res_b.results[c]["outT"].astype(np.float32).T
    if os.environ.get("KTIME"):
        print(f"[kernel] prep1={t1-t0:.2f}s runP={t2-t1:.2f}s prep2={t3-t2:.2f}s "
              f"runA={t4-t3:.2f}s prep3={t5-t4:.2f}s runB={t6-t5:.2f}s")
    return out


# revision 60
# speedup vs baseline: 4.6445x; 1.0005x over previous
"""ClusterAttention Trainium2 kernel — 3-phase design.

Phase P (proj): token-order qkv projection, shared across heads.
  Each core handles (b, token-half): qkv[1152, 4096] = W^T-chunks.T @ featT,
  o-major, bf16 in/out, fp32 psum. Host pre-scales Wq rows by softmax scale.
Host gather: per (b,h) row, gather q/k/v columns into cluster order, build
  augmented q/k (20 rows: 16 qk dims + bias/pos-bias fold rows) and t-major
  v with a ones column (softmax denominator via matmul).
Phase A (attention): per core 6 rows; per cluster S'=k_aug.T@q_aug ->
  exp on ACT (psum->sbuf bf16) -> transposed AV: O[c,i] = sum_j v_t[j,c]E[j,i]
  with 256-wide moving dim; row 64 of O = denominator. Out o-major, bf16,
  unnormalized (host divides by denominator).
Host scatter: normalize, scatter to token order, build feat2T per (b, half).
Phase B (proj): outT[384, 4096] = w_proj chunks.T @ feat2T + bias.

All matmuls bf16 (1 cycle/col vs 4 for fp32); psum->sbuf copies spread over
DVE and ACT; big out DMAs ride the idle Pool SWDGE queue; all DMAs keep
>=512B contiguous runs (below that the model halves DMA bandwidth).
"""
import os
import numpy as np
import ml_dtypes

import concourse.bacc as bacc
import concourse.tile as tile
from concourse import mybir
from concourse.bass_utils import run_bass_kernel_spmd

B, N, C, H, D, K, M = 4, 8192, 384, 12, 2, 32, 256
CH = C // H // 2            # 16
BH = B * H                  # 48
R = BH // 8                 # 6 rows (heads) per core in phase A
SCALE = float((C // H) ** -0.5)
TPB = N * B // 8            # 4096 tokens per core in phases P and B

F32 = mybir.dt.float32
BF16 = mybir.dt.bfloat16
NPBF = ml_dtypes.bfloat16
EXP = mybir.ActivationFunctionType.Exp
# copy-engine set: d=DVE, s=ACT(scalar), g=Pool(gpsimd, breaks device lowering);
# round-robined in order
KCOPY = os.environ.get("KCOPY", "sd")
BOUT = os.environ.get("BOUT", "bf16")  # phase-B output dtype


def _copy_psum(nc, dst, src, i, eng=None):
    eng = eng or KCOPY
    c = eng[i % len(eng)]
    if c == "d":
        nc.vector.tensor_copy(dst, src)
    elif c == "s":
        nc.scalar.copy(dst, src)
    else:
        nc.gpsimd.tensor_copy(dst, src)


def build_phase_p():
    nc = bacc.Bacc(None, target_bir_lowering=False)
    ft = nc.dram_tensor("ft", [3 * 128, TPB], BF16, kind="ExternalInput")
    wt = nc.dram_tensor("wt", [3 * 128, 9 * 128], BF16, kind="ExternalInput")
    qkv = nc.dram_tensor("qkv", [9 * 128, TPB], BF16, kind="ExternalOutput")
    with tile.TileContext(nc) as tc:
        with (
            tc.tile_pool(name="sb", bufs=1) as pool,
            tc.tile_pool(name="sb_o", bufs=1) as p_o,
            tc.tile_pool(name="ps", bufs=4, space="PSUM") as ps,
        ):
            ft_sb = pool.tile([128, 3 * TPB], BF16, tag="ft")
            w_sb = pool.tile([128, 3 * 1152], BF16, tag="w")
            warm = pool.tile([20, 256], BF16, tag="warm")
            nc.vector.memset(warm[:, :], 0.0)
            wps = ps.tile([128, 512], F32, tag="p")
            for i in range(10):
                nc.tensor.matmul(wps[:, 0:256], warm[:, 0:128], warm[:, 0:256],
                                 start=True, stop=True)
            # fused DMAs (3 cc chunks in one 3D access pattern each)
            ftr = ft.rearrange("(c p) t -> p c t", p=128)
            ftv = ft_sb.rearrange("p (c t) -> p c t", t=TPB)
            wtr = wt.rearrange("(c p) j -> p c j", p=128)
            wtv = w_sb.rearrange("p (c j) -> p c j", j=1152)
            # oc=0 weight slice first so the first matmul isn't gated on all of w
            nc.sync.dma_start(wtv[:, :, 0:256], wtr[:, :, 0:256])
            nc.sync.dma_start(ftv[:, :, 0:512], ftr[:, :, 0:512])
            nc.sync.dma_start(wtv[:, :, 256:640], wtr[:, :, 256:640])
            nc.sync.dma_start(wtv[:, :, 640:], wtr[:, :, 640:])
            for q0 in range(512, TPB, 512):
                nc.sync.dma_start(ftv[:, :, q0:q0 + 512], ftr[:, :, q0:q0 + 512])
            o_all = pool.tile([128, 9 * TPB], BF16, tag="o_all")
            o_v = o_all.rearrange("p (c t) -> p c t", t=TPB)
            qkv_v = qkv.rearrange("(c p) t -> p c t", p=128)
            # token tiles taper at the end so the final drain DMA is tiny;
            # fused 3D out DMAs (all 9 oc at once) keep the HWDGE count low
            widths = [512] * 7 + [256, 256]
            drains = [(0, 1024), (1024, 1024), (2048, 1024), (3072, 512),
                      (3584, 256), (3840, 256)]
            t0 = 0
            di = 0
            for tt, w in enumerate(widths):
                for oc in range(9):
                    p = ps.tile([128, 512], F32, tag="p")
                    for cc in range(3):
                        nc.tensor.matmul(
                            p[:, 0:w],
                            w_sb[:, cc * 1152 + oc * 128: cc * 1152 + (oc + 1) * 128],
                            ft_sb[:, cc * TPB + t0: cc * TPB + t0 + w],
                            start=(cc == 0), stop=(cc == 2))
                    _copy_psum(nc, o_v[:, oc, t0:t0 + w], p[:, 0:w], oc)
                t0 += w
                while di < len(drains) and drains[di][0] + drains[di][1] <= t0:
                    d0, dw = drains[di]
                    di += 1
                    nc.sync.dma_start(qkv_v[:, :, d0:d0 + dw],
                                      o_v[:, :, d0:d0 + dw])
    nc.compile()
    return nc


def build_phase_a():
    nc = bacc.Bacc(None, target_bir_lowering=False)
    qk = nc.dram_tensor("qk", [R * 40, N], BF16, kind="ExternalInput")
    vt = nc.dram_tensor("vt", [R * 128, 64 * 65], BF16, kind="ExternalInput")
    og = nc.dram_tensor("og", [R * 65, N], BF16, kind="ExternalOutput")
    with tile.TileContext(nc) as tc:
        with (
            tc.tile_pool(name="row", bufs=2) as p_row,
            tc.tile_pool(name="e", bufs=int(os.environ.get("KEB", "5"))) as p_e,
            tc.tile_pool(name="ps_s", bufs=(3 if os.environ.get("KGRP", "3f") == "2" else 2),
                         space="PSUM") as ps_s,
            tc.tile_pool(name="ps_o", bufs=2, space="PSUM") as ps_o,
        ):
            NP_ = K // 2  # cluster pairs per row
            rows = {}

            # warm the ACT exp table at t=0 so the 1.3us load hides under DMA,
            # and run dummy matmuls so the PE p-state ramps while DMAs fly
            scratch = p_e.tile([1, 8], F32, tag="warm")
            nc.vector.memset(scratch[:, :], 0.0)
            nc.scalar.activation(scratch[:, 4:8], scratch[:, 0:4], EXP)
            wsb = p_e.tile([20, 256], BF16, tag="wsb")
            nc.vector.memset(wsb[:, :], 0.0)
            wps = ps_o.tile([128, 512], F32, tag="po")
            for i in range(14):
                nc.tensor.matmul(wps[:, 0:256], wsb[:, 0:128], wsb[:, 0:256],
                                 start=True, stop=True)

            def load_row(r):
                q_sb = p_row.tile([20, N], BF16, tag="q")
                k_sb = p_row.tile([20, N], BF16, tag="k")
                v_sb = p_row.tile([128, 64 * 65], BF16, tag="v")
                o_sb = p_row.tile([65, N], BF16, tag="o")
                # chunked so the first clusters' operands land early
                for lo, hi in ((0, 256), (256, 2048), (2048, N)):
                    nc.sync.dma_start(q_sb[:, lo:hi], qk[r * 40: r * 40 + 20, lo:hi])
                    nc.sync.dma_start(k_sb[:, lo:hi],
                                      qk[r * 40 + 20: r * 40 + 40, lo:hi])
                nc.sync.dma_start(v_sb[:, 0: 8 * 65],
                                  vt[r * 128:(r + 1) * 128, 0: 8 * 65])
                nc.sync.dma_start(v_sb[:, 8 * 65:],
                                  vt[r * 128:(r + 1) * 128, 8 * 65:])
                rows[r] = (q_sb, k_sb, v_sb.rearrange("p (c w) -> p c w", w=65), o_sb)

            # Flat software pipeline over all clusters: QK+exp runs DEPTH_C
            # clusters ahead of AV+copy so ACT (the bottleneck) never starves.
            # S/E tiles batch 3 clusters ([128, 1536], 3 psum banks) to cut the
            # per-activation init overhead; AV works in 2-cluster units.
            # Input DMAs prefetch one row ahead on SP; out DMAs stream via the
            # idle Pool SWDGE queue, except each row's last two chunks which
            # ride SP/inline so the program tail is one small hop.
            DEPTH_C = int(os.environ.get("KDC", "9"))
            _g = os.environ.get("KGRP", "3f")
            GROUPS = {"3": [3] * 10 + [2], "3f": [2] + [3] * 10, "2": [2] * 16}[_g]
            sitems = []
            for r in range(R):
                c0 = 0
                for g in GROUPS:
                    sitems.append((r, c0, g))
                    c0 += g
            e_of = {}
            av_list = [(r, pp) for r in range(R) for pp in range(NP_)]
            av_ptr = 0
            issued = 0
            done = 0

            def do_av(eng="d"):
                nonlocal av_ptr, done
                qr, qp = av_list[av_ptr]
                av_ptr += 1
                done += 2
                _, _, v_view, o_sb = rows[qr]
                po = ps_o.tile([128, 512], F32, tag="po")
                for u in range(2):
                    et, off = e_of.pop((qr, qp * 2 + u))
                    for jc in range(2):
                        nc.tensor.matmul(
                            po[0:65, u * 256:(u + 1) * 256],
                            v_view[:, (qp * 2 + u) * 2 + jc, 0:65],
                            et[:, off + jc * 256: off + (jc + 1) * 256],
                            start=(jc == 0), stop=(jc == 1))
                _copy_psum(nc, o_sb[:, qp * 512:(qp + 1) * 512],
                           po[0:65, :], 0, eng=eng)
                # stream the row's output: big chunks via Pool SWDGE, the last
                # two pairs per-chunk on SP right behind their copies
                if qp == 7:
                    nc.gpsimd.dma_start(og[qr * 65:(qr + 1) * 65, 0:4096],
                                        o_sb[:, 0:4096])
                elif qp == 13:
                    nc.gpsimd.dma_start(og[qr * 65:(qr + 1) * 65, 4096:7168],
                                        o_sb[:, 4096:7168])
                elif qp >= 14:
                    nc.sync.dma_start(
                        og[qr * 65:(qr + 1) * 65, qp * 512:(qp + 1) * 512],
                        o_sb[:, qp * 512:(qp + 1) * 512])

            load_row(0)
            for r, c0, g in sitems:
                if c0 == 0 and r + 1 < R:
                    load_row(r + 1)
                # drain AV work first so PE has queued work while the next
                # S-group's psum recycles through the pending exp; taper the
                # lag over the last groups so the end-flush backlog is small.
                # At a row's first group the QK matmuls are the critical path
                # (fresh row data), so drain after issuing instead.
                lag = DEPTH_C + g
                left = len(sitems) - sitems.index((r, c0, g))
                if left <= 3:
                    lag = min(lag, 2 * left)
                if c0 > 0:
                    while av_ptr < len(av_list) and issued - done >= lag:
                        do_av()
                q_sb, k_sb, v_view, o_sb = rows[r]
                ps = ps_s.tile([128, 512 * g], F32, tag="s",
                               padded_shape=[128, 512 * max(GROUPS)])
                for i in range(g):
                    col = (c0 + i) * 256
                    nc.tensor.matmul(ps[:, i * 512: i * 512 + 256],
                                     k_sb[:, col:col + 128],
                                     q_sb[:, col:col + 256],
                                     start=True, stop=True)
                    nc.tensor.matmul(ps[:, i * 512 + 256: i * 512 + 512],
                                     k_sb[:, col + 128:col + 256],
                                     q_sb[:, col:col + 256],
                                     start=True, stop=True)
                e = p_e.tile([128, 512 * g], BF16, tag="e",
                             padded_shape=[128, 512 * max(GROUPS)])
                nc.scalar.activation(e[:, :], ps[:, :], EXP)
                for i in range(g):
                    e_of[(r, c0 + i)] = (e, i * 512)
                issued += g
                if c0 == 0:
                    while av_ptr < len(av_list) and issued - done >= lag:
                        do_av()
            flush_i = 0
            while av_ptr < len(av_list):
                do_av(eng="ds"[flush_i % 2])
                flush_i += 1
    nc.compile()
    return nc


def build_phase_b():
    dt_out = F32 if BOUT == "f32" else BF16
    nc = bacc.Bacc(None, target_bir_lowering=False)
    f2T = nc.dram_tensor("f2T", [6 * 128, TPB], BF16, kind="ExternalInput")
    wp2 = nc.dram_tensor("wp2", [6 * 128, 384], BF16, kind="ExternalInput")
    bias = nc.dram_tensor("bias", [3 * 128, 1], F32, kind="ExternalInput")
    outT = nc.dram_tensor("outT", [3 * 128, TPB], dt_out, kind="ExternalOutput")
    with tile.TileContext(nc) as tc:
        with (
            tc.tile_pool(name="sb", bufs=1) as pool,
            tc.tile_pool(name="sb_o", bufs=1) as p_o,
            tc.tile_pool(name="ps", bufs=4, space="PSUM") as ps,
        ):
            fsb = pool.tile([128, 6 * TPB], BF16, tag="fsb")
            wsb = pool.tile([128, 6 * 384], BF16, tag="wsb")
            bsb = pool.tile([128, 3], F32, tag="bsb")
            warm = pool.tile([20, 256], BF16, tag="warm")
            nc.vector.memset(warm[:, :], 0.0)
            wps = ps.tile([128, 512], F32, tag="p")
            for i in range(12):
                nc.tensor.matmul(wps[:, 0:256], warm[:, 0:128], warm[:, 0:256],
                                 start=True, stop=True)
            # fused DMAs (6 cc chunks in one 3D access pattern each)
            nc.sync.dma_start(
                bsb.rearrange("p (c j) -> p c j", j=1)[:, :, :],
                bias.rearrange("(c p) j -> p c j", p=128)[:, :, :])
            fr = f2T.rearrange("(c p) t -> p c t", p=128)
            fv = fsb.rearrange("p (c t) -> p c t", t=TPB)
            wr = wp2.rearrange("(c p) j -> p c j", p=128)
            wv = wsb.rearrange("p (c j) -> p c j", j=384)
            nc.sync.dma_start(wv[:, :, 0:256], wr[:, :, 0:256])
            nc.sync.dma_start(fv[:, :, 0:256], fr[:, :, 0:256])
            nc.sync.dma_start(wv[:, :, 256:], wr[:, :, 256:])
            nc.sync.dma_start(fv[:, :, 256:512], fr[:, :, 256:512])
            for q0 in range(512, TPB, 512):
                nc.sync.dma_start(fv[:, :, q0:q0 + 512], fr[:, :, q0:q0 + 512])
            o_all = pool.tile([128, 3 * TPB], dt_out, tag="o_all")
            o_v = o_all.rearrange("p (c t) -> p c t", t=TPB)
            out_v = outT.rearrange("(c p) t -> p c t", p=128)
            widths = [256, 256] + [512] * 6 + [256, 256]
            drains = [(0, 512), (512, 1024), (1536, 1024), (2560, 1024),
                      (3584, 256), (3840, 256)]
            t0 = 0
            di = 0
            for tt, w in enumerate(widths):
                for oc in range(3):
                    p = ps.tile([128, 512], F32, tag="p")
                    for cc in range(6):
                        nc.tensor.matmul(
                            p[:, 0:w],
                            wsb[:, cc * 384 + oc * 128: cc * 384 + (oc + 1) * 128],
                            fsb[:, cc * TPB + t0: cc * TPB + t0 + w],
                            start=(cc == 0), stop=(cc == 5))
                    nc.vector.tensor_scalar(o_v[:, oc, t0:t0 + w], p[:, 0:w],
                                            bsb[:, oc:oc + 1], None,
                                            mybir.AluOpType.add)
                t0 += w
                while di < len(drains) and drains[di][0] + drains[di][1] <= t0:
                    d0, dw = drains[di]
                    di += 1
                    nc.sync.dma_start(out_v[:, :, d0:d0 + dw],
                                      o_v[:, :, d0:d0 + dw])
    nc.compile()
    return nc


_CACHE = {}
PHASES = ("p", "a", "b")
_BUILDERS = {"p": build_phase_p, "a": build_phase_a, "b": build_phase_b}


def _get(name):
    if name not in _CACHE:
        _CACHE[name] = _BUILDERS[name]()
    return _CACHE[name]


def kernel(pos, feat, member_idx, w_qkv, b_qkv, w_pos, b_pos, w_proj, b_proj):
    import time
    pos = np.asarray(pos, np.float32)
    feat = np.asarray(feat, np.float32)
    mf = np.asarray(member_idx).astype(np.int64).reshape(BH, N)
    w_qkv = np.asarray(w_qkv, np.float32); b_qkv = np.asarray(b_qkv, np.float32)
    w_pos = np.asarray(w_pos, np.float32); b_pos = np.asarray(b_pos, np.float32)
    w_proj = np.asarray(w_proj, np.float32); b_proj = np.asarray(b_proj, np.float32)

    t0 = time.time()
    # ---- phase P host prep: featT per (b, half), prescaled w_qkv^T
    featT = np.ascontiguousarray(feat.transpose(0, 2, 1)).astype(NPBF)  # [B,C,N]
    w_s = w_qkv.copy()
    for h in range(H):
        w_s[h * 96: h * 96 + 16] *= SCALE          # fold softmax scale into Wq
    wt = np.ascontiguousarray(w_s.T).astype(NPBF)  # [384, 1152]
    in_p = []
    for c in range(8):
        b, half = divmod(c, 2)
        in_p.append({"ft": np.ascontiguousarray(featT[b][:, half * TPB:(half + 1) * TPB]),
                     "wt": wt})
    t1 = time.time()
    res_p = run_bass_kernel_spmd(_get("p"), in_p, core_ids=list(range(8)))
    t2 = time.time()

    # ---- host gather into cluster order + augmented rows
    qkv_all = [np.concatenate([res_p.results[2 * b]["qkv"],
                               res_p.results[2 * b + 1]["qkv"]], axis=1)
               for b in range(B)]                  # [1152, N] bf16 each
    pos_n = pos / pos.reshape(-1, D).max(0)
    b_of = np.repeat(np.arange(B), H)
    pos_g = np.take_along_axis(pos_n[b_of], mf[:, :, None], axis=1)   # [48,N,2]
    s_g = np.einsum('rnd,rd->rn', pos_g, np.tile(w_pos, (B, 1))).astype(np.float32)

    ones = np.ones((N,), NPBF)
    zeros = np.zeros((N,), NPBF)
    has_bias = bool(np.any(b_qkv))
    qk_host = np.empty((8, R * 40, N), NPBF)
    vt_host = np.empty((8, R * 128, 64 * 65), NPBF)
    for r in range(BH):
        b, h = divmod(r, H)
        core, rr = divmod(r, R)
        blk = qkv_all[b]
        idx = mf[r]
        qg = blk[h * 96: h * 96 + 16][:, idx]
        kg = blk[h * 96 + 16: h * 96 + 32][:, idx]
        vg = blk[h * 96 + 32: h * 96 + 96][:, idx]          # [64, N] bf16
        row2 = (-s_g[r]).astype(NPBF)
        row5 = (s_g[r] + b_pos[h]).astype(NPBF)
        rowqA, rowkB = zeros, zeros
        if has_bias:
            bq = b_qkv[h * 96: h * 96 + 16]
            bk = b_qkv[h * 96 + 16: h * 96 + 32]
            # qg already carries SCALE, so bk @ qg == scale*(bk . q_raw)
            rowqA = (bk @ qg.astype(np.float32)).astype(NPBF)
            row5 = (s_g[r] + b_pos[h] + SCALE * (bq @ kg.astype(np.float32))
                    + SCALE * float(bq @ bk)).astype(NPBF)
        qa = qk_host[core, rr * 40: rr * 40 + 20]
        qa[0:16] = qg; qa[16] = rowqA; qa[17] = ones; qa[18] = row2; qa[19] = ones
        ka = qk_host[core, rr * 40 + 20: rr * 40 + 40]
        ka[0:16] = kg; ka[16] = ones; ka[17] = row5; ka[18] = ones; ka[19] = rowkB
        vt = np.empty((N, 65), NPBF)
        vt[:, 0:64] = vg.T
        if has_bias:
            bv = np.concatenate([b_qkv[h * 96 + 32: h * 96 + 96]])
            vt[:, 0:64] = (vt[:, 0:64].astype(np.float32) + bv).astype(NPBF)
        vt[:, 64] = 1.0
        vt_host[core, rr * 128:(rr + 1) * 128] = (
            vt.reshape(64, 128, 65).transpose(1, 0, 2).reshape(128, 64 * 65))
    in_a = [{"qk": qk_host[c], "vt": vt_host[c]} for c in range(8)]
    t3 = time.time()
    res_a = run_bass_kernel_spmd(_get("a"), in_a, core_ids=list(range(8)))
    t4 = time.time()

    # ---- host: normalize, scatter to token order, build f2T
    f2T = np.empty((B, 2 * C, N), NPBF)
    for r in range(BH):
        b, h = divmod(r, H)
        core, rr = divmod(r, R)
        o = res_a.results[core]["og"][rr * 65:(rr + 1) * 65].astype(np.float32)
        on = o[0:64] / o[64:65]
        f2T[b][h * 64:(h + 1) * 64][:, mf[r]] = on.astype(NPBF)
    wp2 = np.ascontiguousarray(w_proj.T).astype(NPBF)       # [768, 384]
    b_eff = b_proj + w_proj @ np.concatenate(
        [b_qkv[h * 96 + 32: h * 96 + 96] for h in range(H)])
    in_b = []
    for c in range(8):
        b, half = divmod(c, 2)
        in_b.append({
            "f2T": np.ascontiguousarray(f2T[b][:, half * TPB:(half + 1) * TPB]),
            "wp2": wp2,
            "bias": b_eff.reshape(384, 1).astype(np.float32),
        })
    t5 = time.time()
    res_b = run_bass_kernel_spmd(_get("b"), in_b, core_ids=list(range(8)))
    t6 = time.time()

    out = np.empty((B, N, C), np.float32)
    for c in range(8):
        b, half = divmod(c, 2)
        out[b, half * TPB:(half + 1) * TPB, :] = \
            res_b.results[c]["outT"].astype(np.float32).T
    if os.environ.get("KTIME"):
        print(f"[kernel] prep1={t1-t0:.2f}s runP={t2-t1:.2f}s prep2={t3-t2:.2f}s "
              f"runA={t4-t3:.2f}s prep3={t5-t4:.2f}s runB={t6-t5:.2f}s")
    return out
